# revision 13
# baseline (speedup 1.0000x reference)
"""Evoformer block Trainium2 kernel — 8-core SPMD.

Sharding: MSA over N_seq rows (8/core); pair over first L axis (32 rows/core)
for bias/tri/row-attention, resharded via AllToAll to second-L shards for
column attention + FFN. Triangle einsums run channel-sharded (16 ch/core)
between two AllToAlls, AlphaFold-style.

Layouts: residual streams are CHANNEL-major SBUF tiles [128 ch, tokens].
LayerNorm: cast->DMA-transpose->bn_stats->ACT normalize->DMA-transpose back,
with LN gamma/beta folded into the following linear's weights on the host.
All matmuls bf16 (weights stationary lhsT [din,dout]), fp32 PSUM accumulate.
"""
import numpy as np
import ml_dtypes

import concourse.bass as bass
import concourse.mybir as mybir
import concourse.tile as tile
from concourse import bacc
from concourse.bass_utils import run_bass_kernel_spmd

f32 = mybir.dt.float32
f32r = mybir.dt.float32r
bf16 = mybir.dt.bfloat16
AF = mybir.ActivationFunctionType
ALU = mybir.AluOpType

NCORES = 8
B, N, L, D, H = 1, 64, 256, 128, 4
HD = D // H
EPS = 1e-5
PR = L // NCORES          # pair rows per core = 32
PT = PR * L               # pair tokens per core = 8192
MN = N // NCORES          # msa rows per core = 8
MT = MN * L               # msa tokens per core = 2048
CH = D // NCORES          # channels per core in einsum shard = 16
GSZ = 512                 # tokens per matmul group
PG = PT // GSZ            # 16 pair groups
RG = [list(range(NCORES))]

_BF = ml_dtypes.bfloat16


# ----------------------------------------------------------------------------
# host-side parameter preprocessing
# ----------------------------------------------------------------------------

def _fold_ln(Wnp, bnp, g, be):
    Wf = np.asarray(g)[:, None] * np.asarray(Wnp)
    cf = np.asarray(be) @ np.asarray(Wnp) + np.asarray(bnp)
    return np.asarray(Wf, np.float32), np.asarray(cf, np.float32)


def _cvec(c):
    """bias vector [dout] -> [min(dout,128), nchunks] column-per-chunk layout."""
    c = np.asarray(c, np.float32).reshape(-1)
    if c.size <= 128:
        return np.ascontiguousarray(c.reshape(1, -1).T)      # [dout, 1]
    nch = c.size // 128
    return np.ascontiguousarray(c.reshape(nch, 128).T)        # [128, nch]


def _prep_params(params):
    P = {}

    def addW(name, W, c):
        W = np.asarray(W, np.float32)
        if W.shape[0] > 128:                                  # [512,128] -> [128,4,128]
            nk = W.shape[0] // 128
            W = W.reshape(nk, 128, W.shape[1]).transpose(1, 0, 2)
        P[name + "_w"] = np.ascontiguousarray(W).astype(_BF)
        P[name + "_c"] = _cvec(c)

    P["bias_w"] = np.ascontiguousarray(np.asarray(params["bias"]["w"], np.float32)).astype(_BF)
    P["bias_c"] = _cvec(params["bias"]["b"])

    def attn(prefix, p):
        g, be = p["norm"]["g"], p["norm"]["b"]
        Wq, cq = _fold_ln(p["qkv"]["w"], p["qkv"]["b"], g, be)
        Wq = Wq.copy(); cq = cq.copy()
        Wq[:, :D] /= np.sqrt(HD)
        cq[:D] /= np.sqrt(HD)
        addW(prefix + "_qkv", Wq, cq)
        addW(prefix + "_proj", p["proj"]["w"], p["proj"]["b"])

    def ffn(prefix, p):
        g, be = p["norm"]["g"], p["norm"]["b"]
        W1, c1 = _fold_ln(p["w1"]["w"], p["w1"]["b"], g, be)
        addW(prefix + "_w1", W1, c1)
        addW(prefix + "_w2", p["w2"]["w"], p["w2"]["b"])

    def tri(prefix, p):
        g, be = p["norm"]["g"], p["norm"]["b"]
        for nm in ("pa", "ga", "pb", "gb", "go"):
            Wf, cf = _fold_ln(p[nm]["w"], p[nm]["b"], g, be)
            addW(f"{prefix}_{nm}", Wf, cf)
        Wpo, cpo = _fold_ln(p["po"]["w"], p["po"]["b"], p["norm_o"]["g"],
                            p["norm_o"]["b"])
        addW(prefix + "_po", Wpo, cpo)

    attn("msa", params["msa_attn"])
    ffn("msaffn", params["msa_ffn"])
    tri("triout", params["tri_out"])
    tri("triin", params["tri_in"])
    attn("prow", params["pair_row"])
    attn("pcol", params["pair_col"])
    ffn("pffn", params["pair_ffn"])
    return P


def _dummy_params():
    def lin(di, do):
        return {"w": np.zeros((di, do), np.float32), "b": np.zeros((do,), np.float32)}

    def ln(d):
        return {"g": np.ones((d,), np.float32), "b": np.zeros((d,), np.float32)}

    def attn(d):
        return {"norm": ln(d), "qkv": lin(d, 3 * d), "proj": lin(d, d)}

    def tri(d):
        return {"norm": ln(d), "pa": lin(d, d), "pb": lin(d, d), "ga": lin(d, d),
                "gb": lin(d, d), "go": lin(d, d), "po": lin(d, d), "norm_o": ln(d)}

    def ffn(d):
        return {"norm": ln(d), "w1": lin(d, 4 * d), "w2": lin(4 * d, d)}

    return {"bias": lin(D, H), "msa_attn": attn(D), "msa_ffn": ffn(D),
            "tri_out": tri(D), "tri_in": tri(D), "pair_row": attn(D),
            "pair_col": attn(D), "pair_ffn": ffn(D)}


# ----------------------------------------------------------------------------
# device program
# ----------------------------------------------------------------------------

class Ctx:
    pass


def _program(nc, tc, io):
    ctx = Ctx()
    ctx.nc, ctx.tc, ctx.io = nc, tc, io

    persist = tc.alloc_tile_pool(name="persist", bufs=1)
    dram = tc.alloc_tile_pool(name="dram", bufs=1, space="DRAM")
    ps = tc.alloc_tile_pool(name="psA", bufs=3, space="PSUM")
    ps2 = tc.alloc_tile_pool(name="psB", bufs=3, space="PSUM")
    ctx.persist = persist
    ctx.ps, ctx.ps2 = ps, ps2

    pair = persist.tile([D, PT], f32, name="pair")
    msa_s = persist.tile([D, MT], f32, name="msa_s")
    nc.sync.dma_start(pair[:], io.pair_in[:])
    nc.sync.dma_start(msa_s[:], io.msa_in[:])
    ctx.msa_stream = msa_s

    W = {}
    for k, t in io.w.items():
        W[k] = persist.tile(list(t.shape), t.dtype, name="w_" + k)
        nc.sync.dma_start(W[k][:], t[:])
    ctx.W = W

    epsk = persist.tile([128, 1], f32, name="epsk")
    nc.vector.memset(epsk[:], EPS)
    ctx.eps = epsk

    cb = Ctx()
    cb.bias_send = dram.tile([NCORES, H, PT], bf16, name="bias_send")
    cb.bias_recv = dram.tile([NCORES, H, PT], bf16, name="bias_recv")
    cb.ab_send = dram.tile([NCORES, 2, CH, PR, L], bf16, name="ab_send")
    cb.ab_recv = dram.tile([NCORES, 2, CH, PR, L], bf16, name="ab_recv")
    cb.x_send = dram.tile([NCORES, CH, PR, L], bf16, name="x_send")
    cb.x_recv = dram.tile([NCORES, CH, PR, L], bf16, name="x_recv")
    cb.ab2_send = dram.tile([NCORES, 2, CH, PR, L], bf16, name="ab2_send")
    cb.ab2_recv = dram.tile([NCORES, 2, CH, PR, L], bf16, name="ab2_recv")
    cb.x2_send = dram.tile([NCORES, CH, PR, L], bf16, name="x2_send")
    cb.x2_recv = dram.tile([NCORES, CH, PR, L], bf16, name="x2_recv")
    cb.pc_send = dram.tile([NCORES, D, PR, PR], f32, name="pc_send")
    cb.pc_recv = dram.tile([NCORES, D, PR, PR], f32, name="pc_recv")
    ctx.cb = cb

    def a2a(src, dst):
        nc.gpsimd.collective_compute("AllToAll", ALU.bypass, ins=[src.opt()],
                                     outs=[dst.opt()], replica_groups=RG)

    # ---- phase B: pair bias head -> broadcast
    pair_bf = persist.tile([D, PT], bf16, name="pair_bf")
    nc.vector.tensor_copy(pair_bf[:], pair[:])
    sbB = tc.alloc_tile_pool(name="sbB", bufs=1)
    bias_loc = sbB.tile([H, PT], bf16, name="bias_loc")
    for g in range(PG):
        sl = slice(g * GSZ, (g + 1) * GSZ)
        pb_ps = ps2.tile([H, GSZ], f32, name="psB_t", tag="psB")
        nc.tensor.matmul(pb_ps[:], W["bias_w"][:], pair_bf[:, sl],
                         start=True, stop=True)
        nc.scalar.activation(out=bias_loc[:, sl], in_=pb_ps[:],
                             func=AF.Identity, bias=W["bias_c"][:, 0:1])
    for c in range(NCORES):
        nc.sync.dma_start(cb.bias_send[c], bias_loc[:])
    a2a(cb.bias_send, cb.bias_recv)
    sbB.release()

    go_gate = persist.tile([D, PT], bf16, name="go_gate")

    # ---- phase T1a: tri_out projections + A2A(a,b)
    sbT = tc.alloc_tile_pool(name="sbT1a", bufs=1)
    ctx.sb = sbT
    xh = _ln(ctx, None, PT, "t1", src_bf16=pair_bf)
    _gated_pair(ctx, xh, "triout", cb.ab_send, go_gate)
    a2a(cb.ab_send, cb.ab_recv)
    sbT.release()

    # ---- MSA path (overlaps the A2A)
    _msa_path(ctx)

    # ---- tri_out einsum + apply
    sbE = tc.alloc_tile_pool(name="sbE1", bufs=1)
    ctx.sb = sbE
    _tri_einsum(ctx, cb.ab_recv, cb.x_send, transpose_b=False, tag="e1")
    a2a(cb.x_send, cb.x_recv)
    _tri_apply(ctx, cb.x_recv, "triout", go_gate, pair, tag="o1")
    sbE.release()

    # ---- tri_in
    sbT2 = tc.alloc_tile_pool(name="sbT2a", bufs=1)
    ctx.sb = sbT2
    xh2 = _ln(ctx, pair, PT, "t2")
    _gated_pair(ctx, xh2, "triin", cb.ab2_send, go_gate)
    a2a(cb.ab2_send, cb.ab2_recv)
    sbT2.release()
    sbE2 = tc.alloc_tile_pool(name="sbE2", bufs=1)
    ctx.sb = sbE2
    _tri_einsum(ctx, cb.ab2_recv, cb.x2_send, transpose_b=True, tag="e2")
    a2a(cb.x2_send, cb.x2_recv)
    _tri_apply(ctx, cb.x2_recv, "triin", go_gate, pair, tag="o2")
    sbE2.release()

    # ---- pair row attention
    sbP = tc.alloc_tile_pool(name="sbPr", bufs=1)
    ctx.sb = sbP
    _attention(ctx, pair, PT, "prow", n_rows=PR, strided=False, bias_tiles=None,
               tag="pr")
    sbP.release()

    # ---- reshard rows -> cols
    pview = pair[:].rearrange("d (i j) -> d i j", i=PR)
    for t in range(NCORES):
        nc.sync.dma_start(cb.pc_send[t], pview[:, :, t * PR:(t + 1) * PR])
    a2a(cb.pc_send, cb.pc_recv)
    pcol = pair  # reuse the pair tile for the column-sharded stream
    for c in range(NCORES):
        nc.sync.dma_start(pcol[:, c * PR * PR:(c + 1) * PR * PR],
                          cb.pc_recv[c].rearrange("d a b -> d (a b)"))

    # ---- column attention + FFN on pcol
    sbC = tc.alloc_tile_pool(name="sbPc", bufs=1)
    ctx.sb = sbC
    _attention(ctx, pcol, PT, "pcol", n_rows=PR, strided=True, bias_tiles=None,
               tag="pc")
    sbC.release()
    sbF = tc.alloc_tile_pool(name="sbPf", bufs=1)
    ctx.sb = sbF
    _ffn(ctx, pcol, PT, "pffn", tag="pf")
    sbF.release()

    nc.sync.dma_start(io.pair_out[:], pcol[:])

    ps2.release()
    ps.release()
    dram.release()
    persist.release()


def _ln(ctx, x_cm, n_tok, tag, src_bf16=None):
    nc, sb = ctx.nc, ctx.sb
    nt = n_tok // 128
    if src_bf16 is None:
        xb = sb.tile([D, n_tok], bf16, name=f"lncast_{tag}", tag="lncast")
        nc.vector.tensor_copy(xb[:], x_cm[:, :n_tok])
    else:
        xb = src_bf16
    xtm = sb.tile([128, nt, D], bf16, name=f"lntm_{tag}", tag="lntm")
    for t in range(nt):
        nc.sync.dma_start(xtm[:, t, :], xb[:, t * 128:(t + 1) * 128], transpose=True)
    mv = sb.tile([128, nt, 2], f32, name=f"lnmv_{tag}", tag="lnmv")
    for t in range(nt):
        stats = sb.tile([128, 6], f32, name=f"lnst_{tag}", tag="lnst")
        nc.vector.bn_stats(out=stats[:], in_=xtm[:, t, :])
        nc.vector.bn_aggr(out=mv[:, t, :], in_=stats[:])
    sd = sb.tile([128, nt], f32, name=f"lnsd_{tag}", tag="lnsd")
    nc.scalar.activation(out=sd[:], in_=mv[:, :, 1], func=AF.Sqrt, bias=ctx.eps[:])
    r = sb.tile([128, nt], f32, name=f"lnr_{tag}", tag="lnr")
    nc.vector.reciprocal(out=r[:], in_=sd[:])
    nb = sb.tile([128, nt], f32, name=f"lnnb_{tag}", tag="lnnb")
    nc.vector.tensor_tensor(out=nb[:], in0=mv[:, :, 0], in1=r[:], op=ALU.mult)
    nc.vector.tensor_scalar_mul(nb[:], nb[:], -1.0)
    for t in range(nt):
        nc.scalar.activation(out=xtm[:, t, :], in_=xtm[:, t, :], func=AF.Identity,
                             bias=nb[:, t:t + 1], scale=r[:, t:t + 1])
    out = sb.tile([D, n_tok], bf16, name=f"lnout_{tag}", tag="lnout")
    for t in range(nt):
        nc.sync.dma_start(out[:, t * 128:(t + 1) * 128], xtm[:, t, :], transpose=True)
    return out


def _linear(ctx, Wt, ct, rhs, out_sl, act=None):
    nc = ctx.nc
    acc = ctx.ps.tile([128, GSZ], f32, name="psA_t", tag="psA")
    nc.tensor.matmul(acc[:Wt.shape[-1], :rhs.shape[-1]], Wt, rhs,
                     start=True, stop=True)
    nc.scalar.activation(out=out_sl, in_=acc[:Wt.shape[-1], :rhs.shape[-1]],
                         func=act or AF.Identity, bias=ct)


def _gated_pair(ctx, xh, pfx, send_buf, go_gate):
    nc, sb, W = ctx.nc, ctx.sb, ctx.W
    a_cm = sb.tile([D, PT], bf16, name=f"a_{pfx}", tag="a_cm")
    b_cm = sb.tile([D, PT], bf16, name=f"b_{pfx}", tag="b_cm")
    for g in range(PG):
        sl = slice(g * GSZ, (g + 1) * GSZ)
        for nm, gnm, dst in (("pa", "ga", a_cm), ("pb", "gb", b_cm)):
            p_t = sb.tile([D, GSZ], bf16, name=f"p_{nm}", tag="tri_p")
            _linear(ctx, W[f"{pfx}_{nm}_w"][:], W[f"{pfx}_{nm}_c"][:, 0:1],
                    xh[:, sl], p_t[:])
            g_t = sb.tile([D, GSZ], bf16, name=f"g_{gnm}", tag="tri_g")
            _linear(ctx, W[f"{pfx}_{gnm}_w"][:], W[f"{pfx}_{gnm}_c"][:, 0:1],
                    xh[:, sl], g_t[:], act=AF.Sigmoid)
            nc.vector.tensor_tensor(out=dst[:, sl], in0=p_t[:], in1=g_t[:],
                                    op=ALU.mult)
        _linear(ctx, W[f"{pfx}_go_w"][:], W[f"{pfx}_go_c"][:, 0:1],
                xh[:, sl], go_gate[:, sl], act=AF.Sigmoid)
    for c in range(NCORES):
        nc.sync.dma_start(
            send_buf[c, 0].rearrange("c p l -> c (p l)"), a_cm[c * CH:(c + 1) * CH, :])
        nc.sync.dma_start(
            send_buf[c, 1].rearrange("c p l -> c (p l)"), b_cm[c * CH:(c + 1) * CH, :])


def _tri_einsum(ctx, recv, x_send, transpose_b, tag):
    nc, sb = ctx.nc, ctx.sb
    aT = sb.tile([128, 2, CH, L], bf16, name=f"aT_{tag}", tag="eins_aT")
    for c in range(NCORES):
        for ch in range(CH):
            for jh in range(2):
                nc.sync.dma_start(aT[:, jh, ch, c * PR:(c + 1) * PR],
                                  recv[c, 0, ch, :, jh * 128:(jh + 1) * 128],
                                  transpose=True)
    bT = sb.tile([128, 2, CH, L], bf16, name=f"bT_{tag}", tag="eins_bT")
    if transpose_b:
        for c in range(NCORES):
            for ch in range(CH):
                for kh in range(2):
                    nc.sync.dma_start(bT[:, kh, ch, c * PR:(c + 1) * PR],
                                      recv[c, 1, ch, :, kh * 128:(kh + 1) * 128],
                                      transpose=True)
    else:
        for c in range(NCORES):
            nc.sync.dma_start(bT[(c % 4) * PR:((c % 4) + 1) * PR, c // 4, :, :],
                              recv[c, 1].rearrange("c p l -> p c l"))
    xl = sb.tile([128, 2, CH, L], bf16, name=f"xl_{tag}", tag="eins_x")
    for ch in range(CH):
        for it in range(2):
            acc = ctx.ps2.tile([128, L], f32, name="psB_t", tag="psB")
            for kh in range(2):
                nc.tensor.matmul(acc[:], aT[:, kh, ch, it * 128:(it + 1) * 128],
                                 bT[:, kh, ch, :], start=(kh == 0), stop=(kh == 1))
            nc.vector.tensor_copy(xl[:, it, ch, :], acc[:])
    for c in range(NCORES):
        nc.sync.dma_start(x_send[c].rearrange("c p l -> p c l"),
                          xl[(c % 4) * PR:((c % 4) + 1) * PR, c // 4, :, :])


def _tri_apply(ctx, x_recv, pfx, go_gate, pair, tag):
    nc, sb, W = ctx.nc, ctx.sb, ctx.W
    x_cm = sb.tile([D, PT], bf16, name=f"xcm_{tag}", tag="xcm")
    for c in range(NCORES):
        nc.sync.dma_start(x_cm[c * CH:(c + 1) * CH, :],
                          x_recv[c].rearrange("c p l -> c (p l)"))
    xo = _ln(ctx, None, PT, f"lno_{tag}", src_bf16=x_cm)
    for g in range(PG):
        sl = slice(g * GSZ, (g + 1) * GSZ)
        acc = ctx.ps.tile([128, GSZ], f32, name="psA_t", tag="psA")
        nc.tensor.matmul(acc[:], W[f"{pfx}_po_w"][:], xo[:, sl],
                         start=True, stop=True)
        upd = sb.tile([D, GSZ], f32, name="po_upd", tag="po_upd")
        nc.vector.scalar_tensor_tensor(
            out=upd[:], in0=acc[:], scalar=W[f"{pfx}_po_c"][:, 0:1],
            op0=ALU.add, op1=ALU.mult, in1=go_gate[:, sl])
        nc.vector.tensor_tensor(out=pair[:, sl], in0=pair[:, sl], in1=upd[:],
                                op=ALU.add)


def _attention(ctx, stream, n_tok, pfx, n_rows, strided, bias_tiles, tag):
    nc, sb, W = ctx.nc, ctx.sb, ctx.W
    ng = n_tok // GSZ
    xh = _ln(ctx, stream, n_tok, f"at_{tag}")
    # full qkv staging; heads 0-2 slice at bases 0/32/64, head 3 relocated
    q_cm = sb.tile([D, n_tok], bf16, name=f"q_{tag}", tag="q_cm")
    k_cm = sb.tile([D, n_tok], bf16, name=f"k_{tag}", tag="k_cm")
    v_cm = sb.tile([D, n_tok], bf16, name=f"v_{tag}", tag="v_cm")
    for g in range(ng):
        sl = slice(g * GSZ, (g + 1) * GSZ)
        for ci, dst in ((0, q_cm), (1, k_cm), (2, v_cm)):
            _linear(ctx, W[f"{pfx}_qkv_w"][:, ci * D:(ci + 1) * D],
                    W[f"{pfx}_qkv_c"][:, ci:ci + 1], xh[:, sl], dst[:, sl])
    att_cm = sb.tile([D, n_tok], bf16, name=f"ao_{tag}", tag="att_cm")
    for i in range(n_rows):
        if not strided:
            def rview(t):
                return t[:, i * L:(i + 1) * L]
        else:
            def rview(t):
                return t[:].rearrange("d (i j) -> d j i", j=PR)[:, i, :]
        if strided:
            # stage strided row into contiguous tiles (DMA needs contiguity)
            q_row = sb.tile([D, L], bf16, name="q_row", tag="q_row")
            k_row = sb.tile([D, L], bf16, name="k_row", tag="k_row")
            v_row = sb.tile([D, L], bf16, name="v_row", tag="v_row")
            nc.vector.tensor_copy(q_row[:], rview(q_cm))
            nc.vector.tensor_copy(k_row[:], rview(k_cm))
            nc.vector.tensor_copy(v_row[:], rview(v_cm))
            qv, kv, vv = q_row, k_row, v_row
        else:
            qv, kv, vv = rview(q_cm), rview(k_cm), rview(v_cm)
        v_tm = sb.tile([128, 2, D], bf16, name="v_tm", tag="v_tm")
        for kt in range(2):
            nc.sync.dma_start(v_tm[:, kt, :], vv[:, kt * 128:(kt + 1) * 128],
                              transpose=True)
        att_row = sb.tile([D, L], bf16, name="att_row", tag="att_row")
        # head 3 operands relocated to base 0 (PE quadrant-3 limitation)
        q3 = sb.tile([HD, L], bf16, name="q3", tag="q3")
        k3 = sb.tile([HD, L], bf16, name="k3", tag="k3")
        nc.sync.dma_start(q3[:], qv[3 * HD:4 * HD, :])
        nc.sync.dma_start(k3[:], kv[3 * HD:4 * HD, :])
        for h in range(H):
            hs = slice(h * HD, (h + 1) * HD)
            if h < 3:
                qh, kh = qv[hs, :], kv[hs, :]
            else:
                qh, kh = q3[:], k3[:]
            pT = sb.tile([128, 2, L], bf16, name="pT", tag="pT")
            for xt in range(2):
                sc = ctx.ps.tile([128, GSZ], f32, name="psA_t", tag="psA")
                nc.tensor.matmul(sc[:, :L], qh[:, xt * 128:(xt + 1) * 128],
                                 kh, start=True, stop=True)
                if bias_tiles is not None:
                    nc.vector.tensor_tensor(out=sc[:, :L], in0=sc[:, :L],
                                            in1=bias_tiles[h][xt][:], op=ALU.add)
                prob = sb.tile([128, L], bf16, name="prob", tag="prob")
                rs = sb.tile([128, 1], f32, name="rs", tag="rs")
                nc.scalar.activation(out=prob[:], in_=sc[:, :L], func=AF.Exp,
                                     accum_out=rs[:])
                nc.vector.reciprocal(out=rs[:], in_=rs[:])
                nc.vector.tensor_scalar_mul(prob[:], prob[:], rs[:])
                for kt in range(2):
                    nc.sync.dma_start(pT[:, kt, xt * 128:(xt + 1) * 128],
                                      prob[:, kt * 128:(kt + 1) * 128],
                                      transpose=True)
            av_ps = ctx.ps2.tile([HD, L], f32, name="psB_t", tag="psB")
            for kt in range(2):
                nc.tensor.matmul(av_ps[:], v_tm[:, kt, hs], pT[:, kt, :],
                                 start=(kt == 0), stop=(kt == 1))
            av_sb = sb.tile([HD, L], bf16, name="av_sb", tag="av_sb")
            nc.scalar.copy(out=av_sb[:], in_=av_ps[:])
            if strided:
                nc.sync.dma_start(att_row[hs, :], av_sb[:])
                if h == H - 1:
                    nc.vector.tensor_copy(rview(att_cm), att_row[:])
            else:
                nc.sync.dma_start(rview(att_cm)[hs, :], av_sb[:])
    for g in range(ng):
        sl = slice(g * GSZ, (g + 1) * GSZ)
        acc = ctx.ps.tile([128, GSZ], f32, name="psA_t", tag="psA")
        nc.tensor.matmul(acc[:], W[f"{pfx}_proj_w"][:], att_cm[:, sl],
                         start=True, stop=True)
        nc.vector.scalar_tensor_tensor(
            out=stream[:, sl], in0=acc[:], scalar=W[f"{pfx}_proj_c"][:, 0:1],
            op0=ALU.add, op1=ALU.add, in1=stream[:, sl])


def _ffn(ctx, stream, n_tok, pfx, tag):
    nc, sb, W = ctx.nc, ctx.sb, ctx.W
    ng = n_tok // GSZ
    xh = _ln(ctx, stream, n_tok, f"ffn_{tag}")
    hid = [sb.tile([D, n_tok], bf16, name=f"hid{kc}_{tag}", tag=f"hid{kc}")
           for kc in range(4)]
    for g in range(ng):
        sl = slice(g * GSZ, (g + 1) * GSZ)
        for kc in range(4):
            _linear(ctx, W[f"{pfx}_w1_w"][:, kc * D:(kc + 1) * D],
                    W[f"{pfx}_w1_c"][:, kc:kc + 1], xh[:, sl], hid[kc][:, sl],
                    act=(AF.Tanh if GELU_SUB[0] else AF.Gelu))
        acc = ctx.ps.tile([128, GSZ], f32, name="psA_t", tag="psA")
        for kc in range(4):
            nc.tensor.matmul(acc[:], W[f"{pfx}_w2_w"][:, kc, :], hid[kc][:, sl],
                             start=(kc == 0), stop=(kc == 3))
        nc.vector.scalar_tensor_tensor(
            out=stream[:, sl], in0=acc[:], scalar=W[f"{pfx}_w2_c"][:, 0:1],
            op0=ALU.add, op1=ALU.add, in1=stream[:, sl])


def _msa_path(ctx):
    nc, cb = ctx.nc, ctx.cb
    sb = ctx.tc.alloc_tile_pool(name="sbM", bufs=1)
    ctx.sb = sb
    bt_f32 = []
    for h in range(H):
        row = []
        for xt in range(2):
            bb = sb.tile([128, L], bf16, name=f"bb_{h}_{xt}", tag="biasb")
            for cc in range(4):
                c = 4 * xt + cc
                nc.sync.dma_start(
                    bb[cc * PR:(cc + 1) * PR, :],
                    cb.bias_recv[c, h].rearrange("(x y) -> x y", y=L))
            bf = sb.tile([128, L], f32, name=f"biasf_{h}_{xt}", tag=f"biasf_{h}_{xt}")
            nc.vector.tensor_copy(bf[:], bb[:])
            row.append(bf)
        bt_f32.append(row)
    _attention(ctx, ctx.msa_stream, MT, "msa", n_rows=MN, strided=False,
               bias_tiles=bt_f32, tag="m1")
    _ffn(ctx, ctx.msa_stream, MT, "msaffn", tag="m2")
    nc.sync.dma_start(ctx.io.msa_out[:], ctx.msa_stream[:])
    sb.release()


GELU_SUB = [False]  # sim-only: replace Gelu (unimplemented in CoreSim) with Tanh


def build():
    nc = bacc.Bacc()
    P = _prep_params(_dummy_params())
    io = Ctx()
    io.pair_in = nc.dram_tensor("pair_in", [D, PT], f32, kind="ExternalInput")
    io.msa_in = nc.dram_tensor("msa_in", [D, MT], f32, kind="ExternalInput")
    io.w = {}
    for k, v in P.items():
        dt = bf16 if v.dtype == _BF else f32
        io.w[k] = nc.dram_tensor(k, list(v.shape), dt, kind="ExternalInput")
    io.pair_out = nc.dram_tensor("pair_out", [D, PT], f32, kind="ExternalOutput")
    io.msa_out = nc.dram_tensor("msa_out", [D, MT], f32, kind="ExternalOutput")

    with tile.TileContext(nc) as tc:
        _program(nc, tc, io)
    nc.finalize()
    return nc


_CACHED = {}


def kernel(msa, pair, params):
    msa = np.asarray(msa)
    pair = np.asarray(pair)
    P = _prep_params(params)

    if "nc" not in _CACHED:
        _CACHED["nc"] = build()
    nc = _CACHED["nc"]

    in_maps = []
    for c in range(NCORES):
        m = {}
        pr = pair[0, c * PR:(c + 1) * PR, :, :]
        m["pair_in"] = np.ascontiguousarray(
            pr.transpose(2, 0, 1).reshape(D, PT)).astype(np.float32)
        ms = msa[0, c * MN:(c + 1) * MN, :, :]
        m["msa_in"] = np.ascontiguousarray(
            ms.transpose(2, 0, 1).reshape(D, MT)).astype(np.float32)
        m.update(P)
        in_maps.append(m)

    res = run_bass_kernel_spmd(nc, in_maps, core_ids=list(range(NCORES)))

    msa_out = np.zeros((B, N, L, D), np.float32)
    pair_out = np.zeros((B, L, L, D), np.float32)
    for c in range(NCORES):
        mo = res.results[c]["msa_out"].reshape(D, MN, L).transpose(1, 2, 0)
        msa_out[0, c * MN:(c + 1) * MN] = mo
        po = res.results[c]["pair_out"].reshape(D, L, PR).transpose(1, 2, 0)
        pair_out[0, :, c * PR:(c + 1) * PR, :] = po
    return msa_out, pair_out


# revision 15
# speedup vs baseline: 2.0352x; 2.0352x over previous
"""Evoformer block Trainium2 kernel — 8-core SPMD.

Sharding: MSA over N_seq rows (8/core); pair over first L axis (32 rows/core)
for bias/tri/row-attention, resharded via AllToAll to second-L shards for
column attention + FFN. Triangle einsums run channel-sharded (16 ch/core)
between two AllToAlls, AlphaFold-style.

Layouts: residual streams are CHANNEL-major SBUF tiles [128 ch, tokens].
LayerNorm: cast->DMA-transpose->bn_stats->ACT normalize->DMA-transpose back,
with LN gamma/beta folded into the following linear's weights on the host.
All matmuls bf16 (weights stationary lhsT [din,dout]), fp32 PSUM accumulate.
"""
import numpy as np
import ml_dtypes

import concourse.bass as bass
import concourse.mybir as mybir
import concourse.tile as tile
from concourse import bacc
from concourse.bass_utils import run_bass_kernel_spmd

f32 = mybir.dt.float32
f32r = mybir.dt.float32r
bf16 = mybir.dt.bfloat16
AF = mybir.ActivationFunctionType
ALU = mybir.AluOpType

NCORES = 8
B, N, L, D, H = 1, 64, 256, 128, 4
HD = D // H
EPS = 1e-5
PR = L // NCORES          # pair rows per core = 32
PT = PR * L               # pair tokens per core = 8192
MN = N // NCORES          # msa rows per core = 8
MT = MN * L               # msa tokens per core = 2048
CH = D // NCORES          # channels per core in einsum shard = 16
GSZ = 512                 # tokens per matmul group
PG = PT // GSZ            # 16 pair groups
RG = [list(range(NCORES))]

_BF = ml_dtypes.bfloat16


# ----------------------------------------------------------------------------
# host-side parameter preprocessing
# ----------------------------------------------------------------------------

def _fold_ln(Wnp, bnp, g, be):
    Wf = np.asarray(g)[:, None] * np.asarray(Wnp)
    cf = np.asarray(be) @ np.asarray(Wnp) + np.asarray(bnp)
    return np.asarray(Wf, np.float32), np.asarray(cf, np.float32)


def _cvec(c):
    """bias vector [dout] -> [min(dout,128), nchunks] column-per-chunk layout."""
    c = np.asarray(c, np.float32).reshape(-1)
    if c.size <= 128:
        return np.ascontiguousarray(c.reshape(1, -1).T)      # [dout, 1]
    nch = c.size // 128
    return np.ascontiguousarray(c.reshape(nch, 128).T)        # [128, nch]


def _prep_params(params):
    P = {}

    def addW(name, W, c):
        W = np.asarray(W, np.float32)
        if W.shape[0] > 128:                                  # [512,128] -> [128,4,128]
            nk = W.shape[0] // 128
            W = W.reshape(nk, 128, W.shape[1]).transpose(1, 0, 2)
        P[name + "_w"] = np.ascontiguousarray(W).astype(_BF)
        P[name + "_c"] = _cvec(c)

    P["bias_w"] = np.ascontiguousarray(np.asarray(params["bias"]["w"], np.float32)).astype(_BF)
    P["bias_c"] = _cvec(params["bias"]["b"])

    def attn(prefix, p):
        g, be = p["norm"]["g"], p["norm"]["b"]
        Wq, cq = _fold_ln(p["qkv"]["w"], p["qkv"]["b"], g, be)
        Wq = Wq.copy(); cq = cq.copy()
        Wq[:, :D] /= np.sqrt(HD)
        cq[:D] /= np.sqrt(HD)
        addW(prefix + "_qkv", Wq, cq)
        P[prefix + "_qkv_ch"] = np.ascontiguousarray(
            cq.reshape(3, 128).T[96:128, :])            # [32, 3] head-3 bias
        Wp = np.asarray(p["proj"]["w"], np.float32)      # [128,128] -> [32,4,128]
        P[prefix + "_proj_w"] = np.ascontiguousarray(
            Wp.reshape(4, 32, 128).transpose(1, 0, 2)).astype(_BF)
        P[prefix + "_proj_c"] = _cvec(p["proj"]["b"])

    def ffn(prefix, p):
        g, be = p["norm"]["g"], p["norm"]["b"]
        W1, c1 = _fold_ln(p["w1"]["w"], p["w1"]["b"], g, be)
        addW(prefix + "_w1", W1, c1)
        addW(prefix + "_w2", p["w2"]["w"], p["w2"]["b"])

    def tri(prefix, p):
        g, be = p["norm"]["g"], p["norm"]["b"]
        for nm in ("pa", "ga", "pb", "gb", "go"):
            Wf, cf = _fold_ln(p[nm]["w"], p[nm]["b"], g, be)
            addW(f"{prefix}_{nm}", Wf, cf)
        Wpo, cpo = _fold_ln(p["po"]["w"], p["po"]["b"], p["norm_o"]["g"],
                            p["norm_o"]["b"])
        addW(prefix + "_po", Wpo, cpo)

    attn("msa", params["msa_attn"])
    ffn("msaffn", params["msa_ffn"])
    tri("triout", params["tri_out"])
    tri("triin", params["tri_in"])
    attn("prow", params["pair_row"])
    attn("pcol", params["pair_col"])
    ffn("pffn", params["pair_ffn"])
    return P


def _dummy_params():
    def lin(di, do):
        return {"w": np.zeros((di, do), np.float32), "b": np.zeros((do,), np.float32)}

    def ln(d):
        return {"g": np.ones((d,), np.float32), "b": np.zeros((d,), np.float32)}

    def attn(d):
        return {"norm": ln(d), "qkv": lin(d, 3 * d), "proj": lin(d, d)}

    def tri(d):
        return {"norm": ln(d), "pa": lin(d, d), "pb": lin(d, d), "ga": lin(d, d),
                "gb": lin(d, d), "go": lin(d, d), "po": lin(d, d), "norm_o": ln(d)}

    def ffn(d):
        return {"norm": ln(d), "w1": lin(d, 4 * d), "w2": lin(4 * d, d)}

    return {"bias": lin(D, H), "msa_attn": attn(D), "msa_ffn": ffn(D),
            "tri_out": tri(D), "tri_in": tri(D), "pair_row": attn(D),
            "pair_col": attn(D), "pair_ffn": ffn(D)}


# ----------------------------------------------------------------------------
# device program
# ----------------------------------------------------------------------------

class Ctx:
    pass


def _program(nc, tc, io):
    ctx = Ctx()
    ctx.nc, ctx.tc, ctx.io = nc, tc, io

    persist = tc.alloc_tile_pool(name="persist", bufs=1)
    dram = tc.alloc_tile_pool(name="dram", bufs=1, space="DRAM")
    ps = tc.alloc_tile_pool(name="psA", bufs=3, space="PSUM")
    ps2 = tc.alloc_tile_pool(name="psB", bufs=3, space="PSUM")
    ctx.persist = persist
    ctx.ps, ctx.ps2 = ps, ps2

    pair = persist.tile([D, PT], f32, name="pair")
    msa_s = persist.tile([D, MT], f32, name="msa_s")
    nc.sync.dma_start(pair[:], io.pair_in[:])
    nc.sync.dma_start(msa_s[:], io.msa_in[:])
    ctx.msa_stream = msa_s

    W = {}
    for k, t in io.w.items():
        W[k] = persist.tile(list(t.shape), t.dtype, name="w_" + k)
        nc.sync.dma_start(W[k][:], t[:])
    ctx.W = W

    epsk = persist.tile([128, 1], f32, name="epsk")
    nc.vector.memset(epsk[:], EPS)
    ctx.eps = epsk

    cb = Ctx()
    cb.bias_send = dram.tile([NCORES, H, PT], bf16, name="bias_send")
    cb.bias_recv = dram.tile([NCORES, H, PT], bf16, name="bias_recv")
    cb.ab_send = dram.tile([NCORES, 2, CH, PR, L], bf16, name="ab_send")
    cb.ab_recv = dram.tile([NCORES, 2, CH, PR, L], bf16, name="ab_recv")
    cb.x_send = dram.tile([NCORES, CH, PR, L], bf16, name="x_send")
    cb.x_recv = dram.tile([NCORES, CH, PR, L], bf16, name="x_recv")
    cb.ab2_send = dram.tile([NCORES, 2, CH, PR, L], bf16, name="ab2_send")
    cb.ab2_recv = dram.tile([NCORES, 2, CH, PR, L], bf16, name="ab2_recv")
    cb.x2_send = dram.tile([NCORES, CH, PR, L], bf16, name="x2_send")
    cb.x2_recv = dram.tile([NCORES, CH, PR, L], bf16, name="x2_recv")
    cb.pc_send = dram.tile([NCORES, D, PR, PR], f32, name="pc_send")
    cb.pc_recv = dram.tile([NCORES, D, PR, PR], f32, name="pc_recv")
    ctx.cb = cb

    def a2a(src, dst):
        nc.gpsimd.collective_compute("AllToAll", ALU.bypass, ins=[src.opt()],
                                     outs=[dst.opt()], replica_groups=RG)

    # ---- phase B: pair bias head -> broadcast
    pbf_pool = tc.alloc_tile_pool(name="pbf", bufs=1)
    pair_bf = pbf_pool.tile([D, PT], bf16, name="pair_bf")
    nc.vector.tensor_copy(pair_bf[:], pair[:])
    sbB = tc.alloc_tile_pool(name="sbB", bufs=1)
    bias_loc = sbB.tile([H, PT], bf16, name="bias_loc")
    for g in range(PG):
        sl = slice(g * GSZ, (g + 1) * GSZ)
        pb_ps = ps2.tile([H, GSZ], f32, name="psB_t", tag="psB")
        nc.tensor.matmul(pb_ps[:], W["bias_w"][:], pair_bf[:, sl],
                         start=True, stop=True)
        nc.scalar.activation(out=bias_loc[:, sl], in_=pb_ps[:],
                             func=AF.Identity, bias=W["bias_c"][:, 0:1])
    for c in range(NCORES):
        nc.sync.dma_start(cb.bias_send[c], bias_loc[:])
    a2a(cb.bias_send, cb.bias_recv)
    sbB.release()

    go_gate = persist.tile([D, PT], bf16, name="go_gate")

    # ---- phase T1a: tri_out projections + A2A(a,b)
    sbT = tc.alloc_tile_pool(name="sbT1a", bufs=1)
    ctx.sb = sbT
    xh = _ln(ctx, None, PT, "t1", src_bf16=pair_bf)
    _gated_pair(ctx, xh, "triout", cb.ab_send, go_gate)
    a2a(cb.ab_send, cb.ab_recv)
    sbT.release()
    pbf_pool.release()

    # ---- MSA path (overlaps the A2A)
    _msa_path(ctx)

    # ---- tri_out einsum + apply
    sbE = tc.alloc_tile_pool(name="sbE1", bufs=1)
    ctx.sb = sbE
    _tri_einsum(ctx, cb.ab_recv, cb.x_send, transpose_b=False, tag="e1")
    a2a(cb.x_send, cb.x_recv)
    _tri_apply(ctx, cb.x_recv, "triout", go_gate, pair, tag="o1")
    sbE.release()

    # ---- tri_in
    sbT2 = tc.alloc_tile_pool(name="sbT2a", bufs=1)
    ctx.sb = sbT2
    xh2 = _ln(ctx, pair, PT, "t2")
    _gated_pair(ctx, xh2, "triin", cb.ab2_send, go_gate)
    a2a(cb.ab2_send, cb.ab2_recv)
    sbT2.release()
    sbE2 = tc.alloc_tile_pool(name="sbE2", bufs=1)
    ctx.sb = sbE2
    _tri_einsum(ctx, cb.ab2_recv, cb.x2_send, transpose_b=True, tag="e2")
    a2a(cb.x2_send, cb.x2_recv)
    _tri_apply(ctx, cb.x2_recv, "triin", go_gate, pair, tag="o2")
    sbE2.release()

    # ---- pair row attention
    sbP = tc.alloc_tile_pool(name="sbPr", bufs=1)
    ctx.sb = sbP
    _attention(ctx, pair, PT, "prow", n_rows=PR, strided=False, bias_tiles=None,
               tag="pr")
    sbP.release()

    # ---- reshard rows -> cols
    pview = pair[:].rearrange("d (i j) -> d i j", i=PR)
    for t in range(NCORES):
        nc.sync.dma_start(cb.pc_send[t], pview[:, :, t * PR:(t + 1) * PR])
    a2a(cb.pc_send, cb.pc_recv)
    pcol = pair  # reuse the pair tile for the column-sharded stream
    for c in range(NCORES):
        nc.sync.dma_start(pcol[:, c * PR * PR:(c + 1) * PR * PR],
                          cb.pc_recv[c].rearrange("d a b -> d (a b)"))

    # ---- column attention + FFN on pcol
    sbC = tc.alloc_tile_pool(name="sbPc", bufs=1)
    ctx.sb = sbC
    _attention(ctx, pcol, PT, "pcol", n_rows=PR, strided=True, bias_tiles=None,
               tag="pc")
    sbC.release()
    sbF = tc.alloc_tile_pool(name="sbPf", bufs=1)
    ctx.sb = sbF
    _ffn(ctx, pcol, PT, "pffn", tag="pf")
    sbF.release()

    nc.sync.dma_start(io.pair_out[:], pcol[:])

    ps2.release()
    ps.release()
    dram.release()
    persist.release()


def _ln(ctx, x_cm, n_tok, tag, src_bf16=None):
    nc, sb = ctx.nc, ctx.sb
    nt = n_tok // 128
    if src_bf16 is None:
        xb = sb.tile([D, n_tok], bf16, name=f"lncast_{tag}", tag="lncast")
        nc.vector.tensor_copy(xb[:], x_cm[:, :n_tok])
    else:
        xb = src_bf16
    xtm = sb.tile([128, nt, D], bf16, name=f"lntm_{tag}", tag="lntm")
    nc.sync.dma_start(xtm[:], xb[:, :n_tok], transpose=True)
    mv = sb.tile([128, nt, 2], f32, name=f"lnmv_{tag}", tag="lnmv")
    for t in range(nt):
        stats = sb.tile([128, 6], f32, name=f"lnst_{tag}", tag="lnst")
        nc.vector.bn_stats(out=stats[:], in_=xtm[:, t, :])
        nc.vector.bn_aggr(out=mv[:, t, :], in_=stats[:])
    sd = sb.tile([128, nt], f32, name=f"lnsd_{tag}", tag="lnsd")
    nc.scalar.activation(out=sd[:], in_=mv[:, :, 1], func=AF.Sqrt, bias=ctx.eps[:])
    r = sb.tile([128, nt], f32, name=f"lnr_{tag}", tag="lnr")
    nc.vector.reciprocal(out=r[:], in_=sd[:])
    nb = sb.tile([128, nt], f32, name=f"lnnb_{tag}", tag="lnnb")
    nc.vector.tensor_tensor(out=nb[:], in0=mv[:, :, 0], in1=r[:], op=ALU.mult)
    nc.vector.tensor_scalar_mul(nb[:], nb[:], -1.0)
    for t in range(nt):
        nc.scalar.activation(out=xtm[:, t, :], in_=xtm[:, t, :], func=AF.Identity,
                             bias=nb[:, t:t + 1], scale=r[:, t:t + 1])
    out = sb.tile([D, n_tok], bf16, name=f"lnout_{tag}", tag="lnout")
    nc.scalar.dma_start(out[:].rearrange("d (t f) -> d t f", f=128),
                        xtm[:].rearrange("p t f -> p (t f)"), transpose=True)
    return out


def _linear(ctx, Wt, ct, rhs, out_sl, act=None):
    nc = ctx.nc
    acc = ctx.ps.tile([128, GSZ], f32, name="psA_t", tag="psA")
    nc.tensor.matmul(acc[:Wt.shape[-1], :rhs.shape[-1]], Wt, rhs,
                     start=True, stop=True)
    nc.scalar.activation(out=out_sl, in_=acc[:Wt.shape[-1], :rhs.shape[-1]],
                         func=act or AF.Identity, bias=ct)


def _gated_pair(ctx, xh, pfx, send_buf, go_gate):
    nc, sb, W = ctx.nc, ctx.sb, ctx.W
    a_cm = sb.tile([D, PT], bf16, name=f"a_{pfx}", tag="a_cm")
    b_cm = sb.tile([D, PT], bf16, name=f"b_{pfx}", tag="b_cm")
    for g in range(PG):
        sl = slice(g * GSZ, (g + 1) * GSZ)
        for nm, gnm, dst in (("pa", "ga", a_cm), ("pb", "gb", b_cm)):
            p_t = sb.tile([D, GSZ], bf16, name=f"p_{nm}", tag="tri_p")
            _linear(ctx, W[f"{pfx}_{nm}_w"][:], W[f"{pfx}_{nm}_c"][:, 0:1],
                    xh[:, sl], p_t[:])
            g_t = sb.tile([D, GSZ], bf16, name=f"g_{gnm}", tag="tri_g")
            _linear(ctx, W[f"{pfx}_{gnm}_w"][:], W[f"{pfx}_{gnm}_c"][:, 0:1],
                    xh[:, sl], g_t[:], act=AF.Sigmoid)
            nc.vector.tensor_tensor(out=dst[:, sl], in0=p_t[:], in1=g_t[:],
                                    op=ALU.mult)
        _linear(ctx, W[f"{pfx}_go_w"][:], W[f"{pfx}_go_c"][:, 0:1],
                xh[:, sl], go_gate[:, sl], act=AF.Sigmoid)
    for c in range(NCORES):
        nc.gpsimd.dma_start(
            send_buf[c, 0].rearrange("c p l -> c (p l)"), a_cm[c * CH:(c + 1) * CH, :])
        nc.gpsimd.dma_start(
            send_buf[c, 1].rearrange("c p l -> c (p l)"), b_cm[c * CH:(c + 1) * CH, :])


def _tri_einsum(ctx, recv, x_send, transpose_b, tag):
    nc, sb = ctx.nc, ctx.sb
    aT = sb.tile([128, 2, CH, L], bf16, name=f"aT_{tag}", tag="eins_aT")
    for c in range(NCORES):
        for ch in range(CH):
            eng = nc.sync if (ch % 2 == 0) else nc.scalar
            eng.dma_start(aT[:, :, ch, c * PR:(c + 1) * PR],
                          recv[c, 0, ch], transpose=True)
    bT = sb.tile([128, 2, CH, L], bf16, name=f"bT_{tag}", tag="eins_bT")
    if transpose_b:
        for c in range(NCORES):
            for ch in range(CH):
                eng = nc.sync if (ch % 2 == 1) else nc.scalar
                eng.dma_start(bT[:, :, ch, c * PR:(c + 1) * PR],
                              recv[c, 1, ch], transpose=True)
    else:
        for c in range(NCORES):
            nc.sync.dma_start(bT[(c % 4) * PR:((c % 4) + 1) * PR, c // 4, :, :],
                              recv[c, 1].rearrange("c p l -> p c l"))
    xl = sb.tile([128, 2, CH, L], bf16, name=f"xl_{tag}", tag="eins_x")
    for ch in range(CH):
        for it in range(2):
            acc = ctx.ps2.tile([128, L], f32, name="psB_t", tag="psB")
            for kh in range(2):
                nc.tensor.matmul(acc[:], aT[:, kh, ch, it * 128:(it + 1) * 128],
                                 bT[:, kh, ch, :], start=(kh == 0), stop=(kh == 1))
            nc.vector.tensor_copy(xl[:, it, ch, :], acc[:])
    for c in range(NCORES):
        nc.gpsimd.dma_start(x_send[c].rearrange("c p l -> p c l"),
                            xl[(c % 4) * PR:((c % 4) + 1) * PR, c // 4, :, :])


def _tri_apply(ctx, x_recv, pfx, go_gate, pair, tag):
    nc, sb, W = ctx.nc, ctx.sb, ctx.W
    x_cm = sb.tile([D, PT], bf16, name=f"xcm_{tag}", tag="xcm")
    for c in range(NCORES):
        nc.gpsimd.dma_start(x_cm[c * CH:(c + 1) * CH, :],
                            x_recv[c].rearrange("c p l -> c (p l)"))
    xo = _ln(ctx, None, PT, f"lno_{tag}", src_bf16=x_cm)
    for g in range(PG):
        sl = slice(g * GSZ, (g + 1) * GSZ)
        acc = ctx.ps.tile([128, GSZ], f32, name="psA_t", tag="psA")
        nc.tensor.matmul(acc[:], W[f"{pfx}_po_w"][:], xo[:, sl],
                         start=True, stop=True)
        upd = sb.tile([D, GSZ], f32, name="po_upd", tag="po_upd")
        nc.vector.scalar_tensor_tensor(
            out=upd[:], in0=acc[:], scalar=W[f"{pfx}_po_c"][:, 0:1],
            op0=ALU.add, op1=ALU.mult, in1=go_gate[:, sl])
        nc.vector.tensor_tensor(out=pair[:, sl], in0=pair[:, sl], in1=upd[:],
                                op=ALU.add)


def _attention(ctx, stream, n_tok, pfx, n_rows, strided, bias_tiles, tag):
    nc, sb, W = ctx.nc, ctx.sb, ctx.W
    ng = n_tok // GSZ
    xh = _ln(ctx, stream, n_tok, f"at_{tag}")
    # heads 0-2 at bases 0/32/64 in qA; head 3 in its own base-0 tile qB
    # (PE cannot read operands based at partition 96)
    qA = sb.tile([96, n_tok], bf16, name=f"qA_{tag}", tag="qA")
    kA = sb.tile([96, n_tok], bf16, name=f"kA_{tag}", tag="kA")
    qkB = sb.tile([HD, 2, n_tok], bf16, name=f"qkB_{tag}", tag="qkB")
    qB, kB = qkB[:, 0], qkB[:, 1]
    v_cm = sb.tile([D, n_tok], bf16, name=f"v_{tag}", tag="v_cm")
    for g in range(ng):
        sl = slice(g * GSZ, (g + 1) * GSZ)
        for ci, dA, dB in ((0, qA[:], qB), (1, kA[:], kB)):
            _linear(ctx, W[f"{pfx}_qkv_w"][:, ci * D:ci * D + 96],
                    W[f"{pfx}_qkv_c"][:96, ci:ci + 1], xh[:, sl], dA[:, sl])
            _linear(ctx, W[f"{pfx}_qkv_w"][:, ci * D + 96:(ci + 1) * D],
                    W[f"{pfx}_qkv_ch"][:, ci:ci + 1], xh[:, sl], dB[:, sl])
        _linear(ctx, W[f"{pfx}_qkv_w"][:, 2 * D:3 * D],
                W[f"{pfx}_qkv_c"][:, 2:3], xh[:, sl], v_cm[:, sl])
    for i in range(n_rows):
        if not strided:
            def rview(t):
                return t[:, i * L:(i + 1) * L]
        else:
            def rview(t):
                if hasattr(t, 'tensor'):
                    return t.rearrange("d (i j) -> d j i", j=PR)[:, i, :]
                return t[:].rearrange("d (i j) -> d j i", j=PR)[:, i, :]
        if strided:
            # stage strided row into contiguous tiles (DMA needs contiguity)
            qAr = sb.tile([96, L], bf16, name="qAr", tag="qAr")
            qBr = sb.tile([HD, L], bf16, name="qBr", tag="qBr")
            kAr = sb.tile([96, L], bf16, name="kAr", tag="kAr")
            kBr = sb.tile([HD, L], bf16, name="kBr", tag="kBr")
            v_row = sb.tile([D, L], bf16, name="v_row", tag="v_row")
            nc.vector.tensor_copy(qAr[:], rview(qA))
            nc.vector.tensor_copy(qBr[:], rview(qB))
            nc.vector.tensor_copy(kAr[:], rview(kA))
            nc.vector.tensor_copy(kBr[:], rview(kB))
            nc.vector.tensor_copy(v_row[:], rview(v_cm))
            qAv, qBv, kAv, kBv, vv = qAr[:], qBr[:], kAr[:], kBr[:], v_row[:]
        else:
            qAv, qBv = rview(qA), rview(qB)
            kAv, kBv, vv = rview(kA), rview(kB), rview(v_cm)
        v_tm = sb.tile([128, 2, D], bf16, name="v_tm", tag="v_tm")
        nc.sync.dma_start(v_tm[:], vv, transpose=True)
        att4 = sb.tile([HD, H, L], bf16, name="att4", tag="att4")
        for h in range(H):
            hs = slice(h * HD, (h + 1) * HD)
            if h < 3:
                qh, kh = qAv[hs, :], kAv[hs, :]
            else:
                qh, kh = qBv, kBv
            prob2 = sb.tile([128, 2, L], bf16, name="prob2", tag="prob2")
            for xt in range(2):
                sc = ctx.ps.tile([128, GSZ], f32, name="psA_t", tag="psA")
                nc.tensor.matmul(sc[:, :L], qh[:, xt * 128:(xt + 1) * 128],
                                 kh, start=True, stop=True)
                if bias_tiles is not None:
                    nc.vector.tensor_tensor(out=sc[:, :L], in0=sc[:, :L],
                                            in1=bias_tiles[h][xt][:], op=ALU.add)
                rs = sb.tile([128, 1], f32, name="rs", tag="rs")
                nc.scalar.activation(out=prob2[:, xt, :], in_=sc[:, :L],
                                     func=AF.Exp, accum_out=rs[:])
                nc.vector.reciprocal(out=rs[:], in_=rs[:])
                nc.vector.tensor_scalar_mul(prob2[:, xt, :], prob2[:, xt, :],
                                            rs[:])
            # pT layout [yk 128, xt 2, kt 2, xq 128]: one batched transpose
            pT = sb.tile([128, 2, 2, 128], bf16, name="pT", tag="pT")
            nc.scalar.dma_start(pT[:], prob2[:].rearrange("p x l -> p (x l)"),
                                transpose=True)
            av_ps = ctx.ps2.tile([HD, L], f32, name="psB_t", tag="psB")
            for kt in range(2):
                nc.tensor.matmul(av_ps[:], v_tm[:, kt, hs],
                                 pT[:, :, kt, :], start=(kt == 0),
                                 stop=(kt == 1))
            nc.scalar.copy(out=att4[:, h, :], in_=av_ps[:])
        acc = ctx.ps.tile([128, GSZ], f32, name="psA_t", tag="psA")
        for h in range(H):
            nc.tensor.matmul(acc[:, :L], W[f"{pfx}_proj_w"][:, h, :],
                             att4[:, h, :], start=(h == 0), stop=(h == H - 1))
        nc.vector.scalar_tensor_tensor(
            out=rview(stream), in0=acc[:, :L],
            scalar=W[f"{pfx}_proj_c"][:, 0:1],
            op0=ALU.add, op1=ALU.add, in1=rview(stream))


def _ffn(ctx, stream, n_tok, pfx, tag):
    nc, sb, W = ctx.nc, ctx.sb, ctx.W
    ng = n_tok // GSZ
    xh = _ln(ctx, stream, n_tok, f"ffn_{tag}")
    hid = [sb.tile([D, n_tok], bf16, name=f"hid{kc}_{tag}", tag=f"hid{kc}")
           for kc in range(4)]
    for g in range(ng):
        sl = slice(g * GSZ, (g + 1) * GSZ)
        for kc in range(4):
            _linear(ctx, W[f"{pfx}_w1_w"][:, kc * D:(kc + 1) * D],
                    W[f"{pfx}_w1_c"][:, kc:kc + 1], xh[:, sl], hid[kc][:, sl],
                    act=(AF.Tanh if GELU_SUB[0] else AF.Gelu))
        acc = ctx.ps.tile([128, GSZ], f32, name="psA_t", tag="psA")
        for kc in range(4):
            nc.tensor.matmul(acc[:], W[f"{pfx}_w2_w"][:, kc, :], hid[kc][:, sl],
                             start=(kc == 0), stop=(kc == 3))
        nc.vector.scalar_tensor_tensor(
            out=stream[:, sl], in0=acc[:], scalar=W[f"{pfx}_w2_c"][:, 0:1],
            op0=ALU.add, op1=ALU.add, in1=stream[:, sl])


def _msa_path(ctx):
    nc, cb = ctx.nc, ctx.cb
    sb = ctx.tc.alloc_tile_pool(name="sbM", bufs=1)
    ctx.sb = sb
    bt_f32 = []
    for h in range(H):
        row = []
        for xt in range(2):
            bb = sb.tile([128, L], bf16, name=f"bb_{h}_{xt}", tag="biasb")
            for cc in range(4):
                c = 4 * xt + cc
                nc.sync.dma_start(
                    bb[cc * PR:(cc + 1) * PR, :],
                    cb.bias_recv[c, h].rearrange("(x y) -> x y", y=L))
            bf = sb.tile([128, L], f32, name=f"biasf_{h}_{xt}", tag=f"biasf_{h}_{xt}")
            nc.vector.tensor_copy(bf[:], bb[:])
            row.append(bf)
        bt_f32.append(row)
    _attention(ctx, ctx.msa_stream, MT, "msa", n_rows=MN, strided=False,
               bias_tiles=bt_f32, tag="m1")
    _ffn(ctx, ctx.msa_stream, MT, "msaffn", tag="m2")
    nc.sync.dma_start(ctx.io.msa_out[:], ctx.msa_stream[:])
    sb.release()


GELU_SUB = [False]  # sim-only: replace Gelu (unimplemented in CoreSim) with Tanh


def build():
    nc = bacc.Bacc()
    P = _prep_params(_dummy_params())
    io = Ctx()
    io.pair_in = nc.dram_tensor("pair_in", [D, PT], f32, kind="ExternalInput")
    io.msa_in = nc.dram_tensor("msa_in", [D, MT], f32, kind="ExternalInput")
    io.w = {}
    for k, v in P.items():
        dt = bf16 if v.dtype == _BF else f32
        io.w[k] = nc.dram_tensor(k, list(v.shape), dt, kind="ExternalInput")
    io.pair_out = nc.dram_tensor("pair_out", [D, PT], f32, kind="ExternalOutput")
    io.msa_out = nc.dram_tensor("msa_out", [D, MT], f32, kind="ExternalOutput")

    with tile.TileContext(nc) as tc:
        _program(nc, tc, io)
    nc.finalize()
    return nc


_CACHED = {}


def kernel(msa, pair, params):
    msa = np.asarray(msa)
    pair = np.asarray(pair)
    P = _prep_params(params)

    if "nc" not in _CACHED:
        _CACHED["nc"] = build()
    nc = _CACHED["nc"]

    in_maps = []
    for c in range(NCORES):
        m = {}
        pr = pair[0, c * PR:(c + 1) * PR, :, :]
        m["pair_in"] = np.ascontiguousarray(
            pr.transpose(2, 0, 1).reshape(D, PT)).astype(np.float32)
        ms = msa[0, c * MN:(c + 1) * MN, :, :]
        m["msa_in"] = np.ascontiguousarray(
            ms.transpose(2, 0, 1).reshape(D, MT)).astype(np.float32)
        m.update(P)
        in_maps.append(m)

    res = run_bass_kernel_spmd(nc, in_maps, core_ids=list(range(NCORES)))

    msa_out = np.zeros((B, N, L, D), np.float32)
    pair_out = np.zeros((B, L, L, D), np.float32)
    for c in range(NCORES):
        mo = res.results[c]["msa_out"].reshape(D, MN, L).transpose(1, 2, 0)
        msa_out[0, c * MN:(c + 1) * MN] = mo
        po = res.results[c]["pair_out"].reshape(D, L, PR).transpose(1, 2, 0)
        pair_out[0, :, c * PR:(c + 1) * PR, :] = po
    return msa_out, pair_out


# revision 16
# speedup vs baseline: 2.0973x; 1.0305x over previous
"""Evoformer block Trainium2 kernel — 8-core SPMD.

Sharding: MSA over N_seq rows (8/core); pair over first L axis (32 rows/core)
for bias/tri/row-attention, resharded via AllToAll to second-L shards for
column attention + FFN. Triangle einsums run channel-sharded (16 ch/core)
between two AllToAlls, AlphaFold-style.

Layouts: residual streams are CHANNEL-major SBUF tiles [128 ch, tokens].
LayerNorm: cast->DMA-transpose->bn_stats->ACT normalize->DMA-transpose back,
with LN gamma/beta folded into the following linear's weights on the host.
All matmuls bf16 (weights stationary lhsT [din,dout]), fp32 PSUM accumulate.
"""
import numpy as np
import ml_dtypes

import concourse.bass as bass
import concourse.mybir as mybir
import concourse.tile as tile
from concourse import bacc
from concourse.bass_utils import run_bass_kernel_spmd

f32 = mybir.dt.float32
f32r = mybir.dt.float32r
bf16 = mybir.dt.bfloat16
AF = mybir.ActivationFunctionType
ALU = mybir.AluOpType

NCORES = 8
B, N, L, D, H = 1, 64, 256, 128, 4
HD = D // H
EPS = 1e-5
PR = L // NCORES          # pair rows per core = 32
PT = PR * L               # pair tokens per core = 8192
MN = N // NCORES          # msa rows per core = 8
MT = MN * L               # msa tokens per core = 2048
CH = D // NCORES          # channels per core in einsum shard = 16
GSZ = 512                 # tokens per matmul group
PG = PT // GSZ            # 16 pair groups
RG = [list(range(NCORES))]

_BF = ml_dtypes.bfloat16


# ----------------------------------------------------------------------------
# host-side parameter preprocessing
# ----------------------------------------------------------------------------

def _fold_ln(Wnp, bnp, g, be):
    Wf = np.asarray(g)[:, None] * np.asarray(Wnp)
    cf = np.asarray(be) @ np.asarray(Wnp) + np.asarray(bnp)
    return np.asarray(Wf, np.float32), np.asarray(cf, np.float32)


def _cvec(c):
    """bias vector [dout] -> [min(dout,128), nchunks] column-per-chunk layout."""
    c = np.asarray(c, np.float32).reshape(-1)
    if c.size <= 128:
        return np.ascontiguousarray(c.reshape(1, -1).T)      # [dout, 1]
    nch = c.size // 128
    return np.ascontiguousarray(c.reshape(nch, 128).T)        # [128, nch]


def _prep_params(params):
    P = {}

    def addW(name, W, c):
        W = np.asarray(W, np.float32)
        if W.shape[0] > 128:                                  # [512,128] -> [128,4,128]
            nk = W.shape[0] // 128
            W = W.reshape(nk, 128, W.shape[1]).transpose(1, 0, 2)
        P[name + "_w"] = np.ascontiguousarray(W).astype(_BF)
        P[name + "_c"] = _cvec(c)

    P["bias_w"] = np.ascontiguousarray(np.asarray(params["bias"]["w"], np.float32)).astype(_BF)
    P["bias_c"] = _cvec(params["bias"]["b"])

    def attn(prefix, p):
        g, be = p["norm"]["g"], p["norm"]["b"]
        Wq, cq = _fold_ln(p["qkv"]["w"], p["qkv"]["b"], g, be)
        Wq = Wq.copy(); cq = cq.copy()
        Wq[:, :D] /= np.sqrt(HD)
        cq[:D] /= np.sqrt(HD)
        addW(prefix + "_qkv", Wq, cq)
        P[prefix + "_qkv_ch"] = np.ascontiguousarray(
            cq.reshape(3, 128).T[96:128, :])            # [32, 3] head-3 bias
        Wp = np.asarray(p["proj"]["w"], np.float32)      # [128,128] -> [32,4,128]
        P[prefix + "_proj_w"] = np.ascontiguousarray(
            Wp.reshape(4, 32, 128).transpose(1, 0, 2)).astype(_BF)
        P[prefix + "_proj_c"] = _cvec(p["proj"]["b"])

    def ffn(prefix, p):
        g, be = p["norm"]["g"], p["norm"]["b"]
        W1, c1 = _fold_ln(p["w1"]["w"], p["w1"]["b"], g, be)
        addW(prefix + "_w1", W1, c1)
        addW(prefix + "_w2", p["w2"]["w"], p["w2"]["b"])

    def tri(prefix, p):
        g, be = p["norm"]["g"], p["norm"]["b"]
        for nm in ("pa", "ga", "pb", "gb", "go"):
            Wf, cf = _fold_ln(p[nm]["w"], p[nm]["b"], g, be)
            addW(f"{prefix}_{nm}", Wf, cf)
        Wpo, cpo = _fold_ln(p["po"]["w"], p["po"]["b"], p["norm_o"]["g"],
                            p["norm_o"]["b"])
        addW(prefix + "_po", Wpo, cpo)

    attn("msa", params["msa_attn"])
    ffn("msaffn", params["msa_ffn"])
    tri("triout", params["tri_out"])
    tri("triin", params["tri_in"])
    attn("prow", params["pair_row"])
    attn("pcol", params["pair_col"])
    ffn("pffn", params["pair_ffn"])
    return P


def _dummy_params():
    def lin(di, do):
        return {"w": np.zeros((di, do), np.float32), "b": np.zeros((do,), np.float32)}

    def ln(d):
        return {"g": np.ones((d,), np.float32), "b": np.zeros((d,), np.float32)}

    def attn(d):
        return {"norm": ln(d), "qkv": lin(d, 3 * d), "proj": lin(d, d)}

    def tri(d):
        return {"norm": ln(d), "pa": lin(d, d), "pb": lin(d, d), "ga": lin(d, d),
                "gb": lin(d, d), "go": lin(d, d), "po": lin(d, d), "norm_o": ln(d)}

    def ffn(d):
        return {"norm": ln(d), "w1": lin(d, 4 * d), "w2": lin(4 * d, d)}

    return {"bias": lin(D, H), "msa_attn": attn(D), "msa_ffn": ffn(D),
            "tri_out": tri(D), "tri_in": tri(D), "pair_row": attn(D),
            "pair_col": attn(D), "pair_ffn": ffn(D)}


# ----------------------------------------------------------------------------
# device program
# ----------------------------------------------------------------------------

class Ctx:
    pass


def _program(nc, tc, io):
    ctx = Ctx()
    ctx.nc, ctx.tc, ctx.io = nc, tc, io

    persist = tc.alloc_tile_pool(name="persist", bufs=1)
    dram = tc.alloc_tile_pool(name="dram", bufs=1, space="DRAM")
    ps = tc.alloc_tile_pool(name="psA", bufs=3, space="PSUM")
    ps2 = tc.alloc_tile_pool(name="psB", bufs=3, space="PSUM")
    ctx.persist = persist
    ctx.ps, ctx.ps2 = ps, ps2

    pair = persist.tile([D, PT], f32, name="pair")
    msa_s = persist.tile([D, MT], f32, name="msa_s")
    nc.sync.dma_start(pair[:], io.pair_in[:])
    nc.sync.dma_start(msa_s[:], io.msa_in[:])
    ctx.msa_stream = msa_s

    W = {}
    for k, t in io.w.items():
        W[k] = persist.tile(list(t.shape), t.dtype, name="w_" + k)
        nc.sync.dma_start(W[k][:], t[:])
    ctx.W = W

    epsk = persist.tile([128, 1], f32, name="epsk")
    nc.vector.memset(epsk[:], EPS)
    ctx.eps = epsk

    cb = Ctx()
    cb.bias_send = dram.tile([NCORES, H, PT], bf16, name="bias_send")
    cb.bias_recv = dram.tile([NCORES, H, PT], bf16, name="bias_recv")
    cb.ab_send = dram.tile([NCORES, 2, CH, PR, L], bf16, name="ab_send")
    cb.ab_recv = dram.tile([NCORES, 2, CH, PR, L], bf16, name="ab_recv")
    cb.x_send = dram.tile([NCORES, CH, PR, L], bf16, name="x_send")
    cb.x_recv = dram.tile([NCORES, CH, PR, L], bf16, name="x_recv")
    cb.ab2_send = dram.tile([NCORES, 2, CH, PR, L], bf16, name="ab2_send")
    cb.ab2_recv = dram.tile([NCORES, 2, CH, PR, L], bf16, name="ab2_recv")
    cb.x2_send = dram.tile([NCORES, CH, PR, L], bf16, name="x2_send")
    cb.x2_recv = dram.tile([NCORES, CH, PR, L], bf16, name="x2_recv")
    cb.pc_send = dram.tile([NCORES, D, PR, PR], f32, name="pc_send")
    cb.pc_recv = dram.tile([NCORES, D, PR, PR], f32, name="pc_recv")
    ctx.cb = cb

    def a2a(src, dst):
        nc.gpsimd.collective_compute("AllToAll", ALU.bypass, ins=[src.opt()],
                                     outs=[dst.opt()], replica_groups=RG)

    # ---- phase B: pair bias head -> broadcast
    pbf_pool = tc.alloc_tile_pool(name="pbf", bufs=1)
    pair_bf = pbf_pool.tile([D, PT], bf16, name="pair_bf")
    nc.vector.tensor_copy(pair_bf[:], pair[:])
    sbB = tc.alloc_tile_pool(name="sbB", bufs=1)
    bias_loc = sbB.tile([H, PT], bf16, name="bias_loc")
    for g in range(PG):
        sl = slice(g * GSZ, (g + 1) * GSZ)
        pb_ps = ps2.tile([H, GSZ], f32, name="psB_t", tag="psB")
        nc.tensor.matmul(pb_ps[:], W["bias_w"][:], pair_bf[:, sl],
                         start=True, stop=True)
        nc.scalar.activation(out=bias_loc[:, sl], in_=pb_ps[:],
                             func=AF.Identity, bias=W["bias_c"][:, 0:1])
    for c in range(NCORES):
        nc.sync.dma_start(cb.bias_send[c], bias_loc[:])
    a2a(cb.bias_send, cb.bias_recv)
    sbB.release()

    go_gate = persist.tile([D, PT], bf16, name="go_gate")

    # ---- phase T1a: tri_out projections + A2A(a,b)
    sbT = tc.alloc_tile_pool(name="sbT1a", bufs=1)
    ctx.sb = sbT
    xh = _ln(ctx, None, PT, "t1", src_bf16=pair_bf)
    _gated_pair(ctx, xh, "triout", cb.ab_send, go_gate)
    a2a(cb.ab_send, cb.ab_recv)
    sbT.release()
    pbf_pool.release()

    # ---- MSA path (overlaps the A2A)
    _msa_path(ctx)

    # ---- tri_out einsum + apply
    sbE = tc.alloc_tile_pool(name="sbE1", bufs=1)
    ctx.sb = sbE
    _tri_einsum(ctx, cb.ab_recv, cb.x_send, transpose_b=False, tag="e1")
    a2a(cb.x_send, cb.x_recv)
    _tri_apply(ctx, cb.x_recv, "triout", go_gate, pair, tag="o1")
    sbE.release()

    # ---- tri_in
    sbT2 = tc.alloc_tile_pool(name="sbT2a", bufs=1)
    ctx.sb = sbT2
    xh2 = _ln(ctx, pair, PT, "t2")
    _gated_pair(ctx, xh2, "triin", cb.ab2_send, go_gate)
    a2a(cb.ab2_send, cb.ab2_recv)
    sbT2.release()
    sbE2 = tc.alloc_tile_pool(name="sbE2", bufs=1)
    ctx.sb = sbE2
    _tri_einsum(ctx, cb.ab2_recv, cb.x2_send, transpose_b=True, tag="e2")
    a2a(cb.x2_send, cb.x2_recv)
    _tri_apply(ctx, cb.x2_recv, "triin", go_gate, pair, tag="o2")
    sbE2.release()

    # ---- pair row attention
    sbP = tc.alloc_tile_pool(name="sbPr", bufs=1)
    ctx.sb = sbP
    _attention(ctx, pair, PT, "prow", n_rows=PR, strided=False, bias_tiles=None,
               tag="pr")
    sbP.release()

    # ---- reshard rows -> cols
    pview = pair[:].rearrange("d (i j) -> d i j", i=PR)
    for t in range(NCORES):
        nc.sync.dma_start(cb.pc_send[t], pview[:, :, t * PR:(t + 1) * PR])
    a2a(cb.pc_send, cb.pc_recv)
    pcol = pair  # reuse the pair tile for the column-sharded stream
    for c in range(NCORES):
        nc.sync.dma_start(pcol[:, c * PR * PR:(c + 1) * PR * PR],
                          cb.pc_recv[c].rearrange("d a b -> d (a b)"))

    # ---- column attention + FFN on pcol
    sbC = tc.alloc_tile_pool(name="sbPc", bufs=1)
    ctx.sb = sbC
    _attention(ctx, pcol, PT, "pcol", n_rows=PR, strided=True, bias_tiles=None,
               tag="pc")
    sbC.release()
    sbF = tc.alloc_tile_pool(name="sbPf", bufs=1)
    ctx.sb = sbF
    _ffn(ctx, pcol, PT, "pffn", tag="pf")
    sbF.release()

    nc.sync.dma_start(io.pair_out[:], pcol[:])

    ps2.release()
    ps.release()
    dram.release()
    persist.release()


def _ln(ctx, x_cm, n_tok, tag, src_bf16=None):
    nc, sb = ctx.nc, ctx.sb
    nt = n_tok // 128
    if src_bf16 is None:
        xb = sb.tile([D, n_tok], bf16, name=f"lncast_{tag}", tag="lncast")
        nc.vector.tensor_copy(xb[:], x_cm[:, :n_tok])
    else:
        xb = src_bf16
    xtm = sb.tile([128, nt, D], bf16, name=f"lntm_{tag}", tag="lntm")
    nc.sync.dma_start(xtm[:], xb[:, :n_tok], transpose=True)
    mv = sb.tile([128, nt, 2], f32, name=f"lnmv_{tag}", tag="lnmv")
    for t in range(nt):
        stats = sb.tile([128, 6], f32, name=f"lnst_{tag}", tag="lnst")
        nc.vector.bn_stats(out=stats[:], in_=xtm[:, t, :])
        nc.vector.bn_aggr(out=mv[:, t, :], in_=stats[:])
    sd = sb.tile([128, nt], f32, name=f"lnsd_{tag}", tag="lnsd")
    nc.scalar.activation(out=sd[:], in_=mv[:, :, 1], func=AF.Sqrt, bias=ctx.eps[:])
    r = sb.tile([128, nt], f32, name=f"lnr_{tag}", tag="lnr")
    nc.vector.reciprocal(out=r[:], in_=sd[:])
    for t in range(nt):
        nc.vector.tensor_scalar(out=xtm[:, t, :], in0=xtm[:, t, :],
                                scalar1=mv[:, t, 0:1], scalar2=r[:, t:t + 1],
                                op0=ALU.subtract, op1=ALU.mult)
    out = sb.tile([D, n_tok], bf16, name=f"lnout_{tag}", tag="lnout")
    nc.scalar.dma_start(out[:].rearrange("d (t f) -> d t f", f=128),
                        xtm[:].rearrange("p t f -> p (t f)"), transpose=True)
    return out


def _linear(ctx, Wt, ct, rhs, out_sl, act=None):
    nc = ctx.nc
    acc = ctx.ps.tile([128, GSZ], f32, name="psA_t", tag="psA")
    nc.tensor.matmul(acc[:Wt.shape[-1], :rhs.shape[-1]], Wt, rhs,
                     start=True, stop=True)
    if act is None:
        nc.vector.tensor_scalar_add(out_sl, acc[:Wt.shape[-1], :rhs.shape[-1]],
                                    ct)
    else:
        nc.scalar.activation(out=out_sl, in_=acc[:Wt.shape[-1], :rhs.shape[-1]],
                             func=act, bias=ct)


def _gated_pair(ctx, xh, pfx, send_buf, go_gate):
    nc, sb, W = ctx.nc, ctx.sb, ctx.W
    a_cm = sb.tile([D, PT], bf16, name=f"a_{pfx}", tag="a_cm")
    b_cm = sb.tile([D, PT], bf16, name=f"b_{pfx}", tag="b_cm")
    for g in range(PG):
        sl = slice(g * GSZ, (g + 1) * GSZ)
        for nm, gnm, dst in (("pa", "ga", a_cm), ("pb", "gb", b_cm)):
            p_t = sb.tile([D, GSZ], bf16, name=f"p_{nm}", tag="tri_p")
            _linear(ctx, W[f"{pfx}_{nm}_w"][:], W[f"{pfx}_{nm}_c"][:, 0:1],
                    xh[:, sl], p_t[:])
            g_t = sb.tile([D, GSZ], bf16, name=f"g_{gnm}", tag="tri_g")
            _linear(ctx, W[f"{pfx}_{gnm}_w"][:], W[f"{pfx}_{gnm}_c"][:, 0:1],
                    xh[:, sl], g_t[:], act=AF.Sigmoid)
            nc.vector.tensor_tensor(out=dst[:, sl], in0=p_t[:], in1=g_t[:],
                                    op=ALU.mult)
        _linear(ctx, W[f"{pfx}_go_w"][:], W[f"{pfx}_go_c"][:, 0:1],
                xh[:, sl], go_gate[:, sl], act=AF.Sigmoid)
    for c in range(NCORES):
        nc.gpsimd.dma_start(
            send_buf[c, 0].rearrange("c p l -> c (p l)"), a_cm[c * CH:(c + 1) * CH, :])
        nc.gpsimd.dma_start(
            send_buf[c, 1].rearrange("c p l -> c (p l)"), b_cm[c * CH:(c + 1) * CH, :])


def _tri_einsum(ctx, recv, x_send, transpose_b, tag):
    nc, sb = ctx.nc, ctx.sb
    aT = sb.tile([128, 2, CH, L], bf16, name=f"aT_{tag}", tag="eins_aT")
    for c in range(NCORES):
        for ch in range(CH):
            eng = nc.sync if (ch % 2 == 0) else nc.scalar
            eng.dma_start(aT[:, :, ch, c * PR:(c + 1) * PR],
                          recv[c, 0, ch], transpose=True)
    bT = sb.tile([128, 2, CH, L], bf16, name=f"bT_{tag}", tag="eins_bT")
    if transpose_b:
        for c in range(NCORES):
            for ch in range(CH):
                eng = nc.sync if (ch % 2 == 1) else nc.scalar
                eng.dma_start(bT[:, :, ch, c * PR:(c + 1) * PR],
                              recv[c, 1, ch], transpose=True)
    else:
        for c in range(NCORES):
            nc.sync.dma_start(bT[(c % 4) * PR:((c % 4) + 1) * PR, c // 4, :, :],
                              recv[c, 1].rearrange("c p l -> p c l"))
    xl = sb.tile([128, 2, CH, L], bf16, name=f"xl_{tag}", tag="eins_x")
    for ch in range(CH):
        for it in range(2):
            acc = ctx.ps2.tile([128, L], f32, name="psB_t", tag="psB")
            for kh in range(2):
                nc.tensor.matmul(acc[:], aT[:, kh, ch, it * 128:(it + 1) * 128],
                                 bT[:, kh, ch, :], start=(kh == 0), stop=(kh == 1))
            nc.vector.tensor_copy(xl[:, it, ch, :], acc[:])
    for c in range(NCORES):
        nc.gpsimd.dma_start(x_send[c].rearrange("c p l -> p c l"),
                            xl[(c % 4) * PR:((c % 4) + 1) * PR, c // 4, :, :])


def _tri_apply(ctx, x_recv, pfx, go_gate, pair, tag):
    nc, sb, W = ctx.nc, ctx.sb, ctx.W
    x_cm = sb.tile([D, PT], bf16, name=f"xcm_{tag}", tag="xcm")
    for c in range(NCORES):
        nc.gpsimd.dma_start(x_cm[c * CH:(c + 1) * CH, :],
                            x_recv[c].rearrange("c p l -> c (p l)"))
    xo = _ln(ctx, None, PT, f"lno_{tag}", src_bf16=x_cm)
    for g in range(PG):
        sl = slice(g * GSZ, (g + 1) * GSZ)
        acc = ctx.ps.tile([128, GSZ], f32, name="psA_t", tag="psA")
        nc.tensor.matmul(acc[:], W[f"{pfx}_po_w"][:], xo[:, sl],
                         start=True, stop=True)
        upd = sb.tile([D, GSZ], f32, name="po_upd", tag="po_upd")
        nc.vector.scalar_tensor_tensor(
            out=upd[:], in0=acc[:], scalar=W[f"{pfx}_po_c"][:, 0:1],
            op0=ALU.add, op1=ALU.mult, in1=go_gate[:, sl])
        nc.vector.tensor_tensor(out=pair[:, sl], in0=pair[:, sl], in1=upd[:],
                                op=ALU.add)


def _attention(ctx, stream, n_tok, pfx, n_rows, strided, bias_tiles, tag):
    nc, sb, W = ctx.nc, ctx.sb, ctx.W
    ng = n_tok // GSZ
    xh = _ln(ctx, stream, n_tok, f"at_{tag}")
    # heads 0-2 at bases 0/32/64 in qA; head 3 in its own base-0 tile qB
    # (PE cannot read operands based at partition 96)
    qA = sb.tile([96, n_tok], bf16, name=f"qA_{tag}", tag="qA")
    kA = sb.tile([96, n_tok], bf16, name=f"kA_{tag}", tag="kA")
    qkB = sb.tile([HD, 2, n_tok], bf16, name=f"qkB_{tag}", tag="qkB")
    qB, kB = qkB[:, 0], qkB[:, 1]
    v_cm = sb.tile([D, n_tok], bf16, name=f"v_{tag}", tag="v_cm")
    for g in range(ng):
        sl = slice(g * GSZ, (g + 1) * GSZ)
        for ci, dA, dB in ((0, qA[:], qB), (1, kA[:], kB)):
            _linear(ctx, W[f"{pfx}_qkv_w"][:, ci * D:ci * D + 96],
                    W[f"{pfx}_qkv_c"][:96, ci:ci + 1], xh[:, sl], dA[:, sl])
            _linear(ctx, W[f"{pfx}_qkv_w"][:, ci * D + 96:(ci + 1) * D],
                    W[f"{pfx}_qkv_ch"][:, ci:ci + 1], xh[:, sl], dB[:, sl])
        _linear(ctx, W[f"{pfx}_qkv_w"][:, 2 * D:3 * D],
                W[f"{pfx}_qkv_c"][:, 2:3], xh[:, sl], v_cm[:, sl])
    for i in range(n_rows):
        if not strided:
            def rview(t):
                return t[:, i * L:(i + 1) * L]
        else:
            def rview(t):
                if hasattr(t, 'tensor'):
                    return t.rearrange("d (i j) -> d j i", j=PR)[:, i, :]
                return t[:].rearrange("d (i j) -> d j i", j=PR)[:, i, :]
        if strided:
            # stage strided row into contiguous tiles (DMA needs contiguity)
            qAr = sb.tile([96, L], bf16, name="qAr", tag="qAr")
            qBr = sb.tile([HD, L], bf16, name="qBr", tag="qBr")
            kAr = sb.tile([96, L], bf16, name="kAr", tag="kAr")
            kBr = sb.tile([HD, L], bf16, name="kBr", tag="kBr")
            v_row = sb.tile([D, L], bf16, name="v_row", tag="v_row")
            nc.vector.tensor_copy(qAr[:], rview(qA))
            nc.vector.tensor_copy(qBr[:], rview(qB))
            nc.vector.tensor_copy(kAr[:], rview(kA))
            nc.vector.tensor_copy(kBr[:], rview(kB))
            nc.vector.tensor_copy(v_row[:], rview(v_cm))
            qAv, qBv, kAv, kBv, vv = qAr[:], qBr[:], kAr[:], kBr[:], v_row[:]
        else:
            qAv, qBv = rview(qA), rview(qB)
            kAv, kBv, vv = rview(kA), rview(kB), rview(v_cm)
        v_tm = sb.tile([128, 2, D], bf16, name="v_tm", tag="v_tm")
        nc.sync.dma_start(v_tm[:], vv, transpose=True)
        att4 = sb.tile([HD, H, L], bf16, name="att4", tag="att4")
        for h in range(H):
            hs = slice(h * HD, (h + 1) * HD)
            if h < 3:
                qh, kh = qAv[hs, :], kAv[hs, :]
            else:
                qh, kh = qBv, kBv
            prob2 = sb.tile([128, 2, L], bf16, name="prob2", tag="prob2")
            for xt in range(2):
                sc = ctx.ps.tile([128, GSZ], f32, name="psA_t", tag="psA")
                nc.tensor.matmul(sc[:, :L], qh[:, xt * 128:(xt + 1) * 128],
                                 kh, start=True, stop=True)
                if bias_tiles is not None:
                    nc.vector.tensor_tensor(out=sc[:, :L], in0=sc[:, :L],
                                            in1=bias_tiles[h][xt][:], op=ALU.add)
                rs = sb.tile([128, 1], f32, name="rs", tag="rs")
                nc.scalar.activation(out=prob2[:, xt, :], in_=sc[:, :L],
                                     func=AF.Exp, accum_out=rs[:])
                nc.vector.reciprocal(out=rs[:], in_=rs[:])
                nc.vector.tensor_scalar_mul(prob2[:, xt, :], prob2[:, xt, :],
                                            rs[:])
            # pT layout [yk 128, xt 2, kt 2, xq 128]: one batched transpose
            pT = sb.tile([128, 2, 2, 128], bf16, name="pT", tag="pT")
            nc.sync.dma_start(pT[:], prob2[:].rearrange("p x l -> p (x l)"),
                               transpose=True)
            av_ps = ctx.ps2.tile([HD, L], f32, name="psB_t", tag="psB")
            for kt in range(2):
                nc.tensor.matmul(av_ps[:], v_tm[:, kt, hs],
                                 pT[:, :, kt, :], start=(kt == 0),
                                 stop=(kt == 1))
            nc.scalar.copy(out=att4[:, h, :], in_=av_ps[:])
        acc = ctx.ps.tile([128, GSZ], f32, name="psA_t", tag="psA")
        for h in range(H):
            nc.tensor.matmul(acc[:, :L], W[f"{pfx}_proj_w"][:, h, :],
                             att4[:, h, :], start=(h == 0), stop=(h == H - 1))
        nc.vector.scalar_tensor_tensor(
            out=rview(stream), in0=acc[:, :L],
            scalar=W[f"{pfx}_proj_c"][:, 0:1],
            op0=ALU.add, op1=ALU.add, in1=rview(stream))


def _ffn(ctx, stream, n_tok, pfx, tag):
    nc, sb, W = ctx.nc, ctx.sb, ctx.W
    ng = n_tok // GSZ
    xh = _ln(ctx, stream, n_tok, f"ffn_{tag}")
    hid = [sb.tile([D, n_tok], bf16, name=f"hid{kc}_{tag}", tag=f"hid{kc}")
           for kc in range(4)]
    for g in range(ng):
        sl = slice(g * GSZ, (g + 1) * GSZ)
        for kc in range(4):
            _linear(ctx, W[f"{pfx}_w1_w"][:, kc * D:(kc + 1) * D],
                    W[f"{pfx}_w1_c"][:, kc:kc + 1], xh[:, sl], hid[kc][:, sl],
                    act=(AF.Tanh if GELU_SUB[0] else AF.Gelu))
        acc = ctx.ps.tile([128, GSZ], f32, name="psA_t", tag="psA")
        for kc in range(4):
            nc.tensor.matmul(acc[:], W[f"{pfx}_w2_w"][:, kc, :], hid[kc][:, sl],
                             start=(kc == 0), stop=(kc == 3))
        nc.vector.scalar_tensor_tensor(
            out=stream[:, sl], in0=acc[:], scalar=W[f"{pfx}_w2_c"][:, 0:1],
            op0=ALU.add, op1=ALU.add, in1=stream[:, sl])


def _msa_path(ctx):
    nc, cb = ctx.nc, ctx.cb
    sb = ctx.tc.alloc_tile_pool(name="sbM", bufs=1)
    ctx.sb = sb
    bt_f32 = []
    for h in range(H):
        row = []
        for xt in range(2):
            bb = sb.tile([128, L], bf16, name=f"bb_{h}_{xt}", tag="biasb")
            for cc in range(4):
                c = 4 * xt + cc
                nc.sync.dma_start(
                    bb[cc * PR:(cc + 1) * PR, :],
                    cb.bias_recv[c, h].rearrange("(x y) -> x y", y=L))
            bf = sb.tile([128, L], f32, name=f"biasf_{h}_{xt}", tag=f"biasf_{h}_{xt}")
            nc.vector.tensor_copy(bf[:], bb[:])
            row.append(bf)
        bt_f32.append(row)
    _attention(ctx, ctx.msa_stream, MT, "msa", n_rows=MN, strided=False,
               bias_tiles=bt_f32, tag="m1")
    _ffn(ctx, ctx.msa_stream, MT, "msaffn", tag="m2")
    nc.sync.dma_start(ctx.io.msa_out[:], ctx.msa_stream[:])
    sb.release()


GELU_SUB = [False]  # sim-only: replace Gelu (unimplemented in CoreSim) with Tanh


def build():
    nc = bacc.Bacc()
    P = _prep_params(_dummy_params())
    io = Ctx()
    io.pair_in = nc.dram_tensor("pair_in", [D, PT], f32, kind="ExternalInput")
    io.msa_in = nc.dram_tensor("msa_in", [D, MT], f32, kind="ExternalInput")
    io.w = {}
    for k, v in P.items():
        dt = bf16 if v.dtype == _BF else f32
        io.w[k] = nc.dram_tensor(k, list(v.shape), dt, kind="ExternalInput")
    io.pair_out = nc.dram_tensor("pair_out", [D, PT], f32, kind="ExternalOutput")
    io.msa_out = nc.dram_tensor("msa_out", [D, MT], f32, kind="ExternalOutput")

    with tile.TileContext(nc) as tc:
        _program(nc, tc, io)
    nc.finalize()
    return nc


_CACHED = {}


def kernel(msa, pair, params):
    msa = np.asarray(msa)
    pair = np.asarray(pair)
    P = _prep_params(params)

    if "nc" not in _CACHED:
        _CACHED["nc"] = build()
    nc = _CACHED["nc"]

    in_maps = []
    for c in range(NCORES):
        m = {}
        pr = pair[0, c * PR:(c + 1) * PR, :, :]
        m["pair_in"] = np.ascontiguousarray(
            pr.transpose(2, 0, 1).reshape(D, PT)).astype(np.float32)
        ms = msa[0, c * MN:(c + 1) * MN, :, :]
        m["msa_in"] = np.ascontiguousarray(
            ms.transpose(2, 0, 1).reshape(D, MT)).astype(np.float32)
        m.update(P)
        in_maps.append(m)

    res = run_bass_kernel_spmd(nc, in_maps, core_ids=list(range(NCORES)))

    msa_out = np.zeros((B, N, L, D), np.float32)
    pair_out = np.zeros((B, L, L, D), np.float32)
    for c in range(NCORES):
        mo = res.results[c]["msa_out"].reshape(D, MN, L).transpose(1, 2, 0)
        msa_out[0, c * MN:(c + 1) * MN] = mo
        po = res.results[c]["pair_out"].reshape(D, L, PR).transpose(1, 2, 0)
        pair_out[0, :, c * PR:(c + 1) * PR, :] = po
    return msa_out, pair_out


# revision 20
# speedup vs baseline: 2.8301x; 1.3494x over previous
"""Evoformer block Trainium2 kernel — 8-core SPMD.

Sharding: MSA over N_seq rows (8/core); pair over first L axis (32 rows/core)
for bias/tri/row-attention, resharded via AllToAll to second-L shards for
column attention + FFN. Triangle einsums run channel-sharded (16 ch/core)
between two AllToAlls, AlphaFold-style.

Layouts: residual streams are CHANNEL-major SBUF tiles [128 ch, tokens].
LayerNorm: cast->DMA-transpose->bn_stats->ACT normalize->DMA-transpose back,
with LN gamma/beta folded into the following linear's weights on the host.
All matmuls bf16 (weights stationary lhsT [din,dout]), fp32 PSUM accumulate.
"""
import numpy as np
import ml_dtypes

import concourse.bass as bass
import concourse.mybir as mybir
import concourse.tile as tile
from concourse import bacc
from concourse.bass_utils import run_bass_kernel_spmd

f32 = mybir.dt.float32
f32r = mybir.dt.float32r
bf16 = mybir.dt.bfloat16
AF = mybir.ActivationFunctionType
ALU = mybir.AluOpType

NCORES = 8
B, N, L, D, H = 1, 64, 256, 128, 4
HD = D // H
EPS = 1e-5
PR = L // NCORES          # pair rows per core = 32
PT = PR * L               # pair tokens per core = 8192
MN = N // NCORES          # msa rows per core = 8
MT = MN * L               # msa tokens per core = 2048
CH = D // NCORES          # channels per core in einsum shard = 16
GSZ = 512                 # tokens per matmul group
PG = PT // GSZ            # 16 pair groups
RG = [list(range(NCORES))]

_BF = ml_dtypes.bfloat16


# ----------------------------------------------------------------------------
# host-side parameter preprocessing
# ----------------------------------------------------------------------------

def _fold_ln(Wnp, bnp, g, be):
    Wf = np.asarray(g)[:, None] * np.asarray(Wnp)
    cf = np.asarray(be) @ np.asarray(Wnp) + np.asarray(bnp)
    return np.asarray(Wf, np.float32), np.asarray(cf, np.float32)


def _cvec(c):
    """bias vector [dout] -> [min(dout,128), nchunks] column-per-chunk layout."""
    c = np.asarray(c, np.float32).reshape(-1)
    if c.size <= 128:
        return np.ascontiguousarray(c.reshape(1, -1).T)      # [dout, 1]
    nch = c.size // 128
    return np.ascontiguousarray(c.reshape(nch, 128).T)        # [128, nch]


def _prep_params(params):
    P = {}

    def addW(name, W, c):
        W = np.asarray(W, np.float32)
        if W.shape[0] > 128:                                  # [512,128] -> [128,4,128]
            nk = W.shape[0] // 128
            W = W.reshape(nk, 128, W.shape[1]).transpose(1, 0, 2)
        P[name + "_w"] = np.ascontiguousarray(W).astype(_BF)
        P[name + "_c"] = _cvec(c)

    P["bias_w"] = np.ascontiguousarray(np.asarray(params["bias"]["w"], np.float32)).astype(_BF)
    P["bias_c"] = _cvec(params["bias"]["b"])

    def attn(prefix, p):
        g, be = p["norm"]["g"], p["norm"]["b"]
        Wq, cq = _fold_ln(p["qkv"]["w"], p["qkv"]["b"], g, be)
        Wq = Wq.copy(); cq = cq.copy()
        Wq[:, :D] /= np.sqrt(HD)
        cq[:D] /= np.sqrt(HD)
        addW(prefix + "_qkv", Wq, cq)
        P[prefix + "_qkv_ch"] = np.ascontiguousarray(
            cq.reshape(3, 128).T[96:128, :])            # [32, 3] head-3 bias
        Wp = np.asarray(p["proj"]["w"], np.float32)      # [128,128] -> [32,4,128]
        P[prefix + "_proj_w"] = np.ascontiguousarray(
            Wp.reshape(4, 32, 128).transpose(1, 0, 2)).astype(_BF)
        P[prefix + "_proj_c"] = _cvec(p["proj"]["b"])

    def ffn(prefix, p):
        g, be = p["norm"]["g"], p["norm"]["b"]
        W1, c1 = _fold_ln(p["w1"]["w"], p["w1"]["b"], g, be)
        addW(prefix + "_w1", W1, c1)
        addW(prefix + "_w2", p["w2"]["w"], p["w2"]["b"])

    def tri(prefix, p):
        g, be = p["norm"]["g"], p["norm"]["b"]
        for nm in ("pa", "ga", "pb", "gb", "go"):
            Wf, cf = _fold_ln(p[nm]["w"], p[nm]["b"], g, be)
            addW(f"{prefix}_{nm}", Wf, cf)
        Wpo, cpo = _fold_ln(p["po"]["w"], p["po"]["b"], p["norm_o"]["g"],
                            p["norm_o"]["b"])
        addW(prefix + "_po", Wpo, cpo)

    attn("msa", params["msa_attn"])
    ffn("msaffn", params["msa_ffn"])
    tri("triout", params["tri_out"])
    tri("triin", params["tri_in"])
    attn("prow", params["pair_row"])
    attn("pcol", params["pair_col"])
    ffn("pffn", params["pair_ffn"])
    return P


def _dummy_params():
    def lin(di, do):
        return {"w": np.zeros((di, do), np.float32), "b": np.zeros((do,), np.float32)}

    def ln(d):
        return {"g": np.ones((d,), np.float32), "b": np.zeros((d,), np.float32)}

    def attn(d):
        return {"norm": ln(d), "qkv": lin(d, 3 * d), "proj": lin(d, d)}

    def tri(d):
        return {"norm": ln(d), "pa": lin(d, d), "pb": lin(d, d), "ga": lin(d, d),
                "gb": lin(d, d), "go": lin(d, d), "po": lin(d, d), "norm_o": ln(d)}

    def ffn(d):
        return {"norm": ln(d), "w1": lin(d, 4 * d), "w2": lin(4 * d, d)}

    return {"bias": lin(D, H), "msa_attn": attn(D), "msa_ffn": ffn(D),
            "tri_out": tri(D), "tri_in": tri(D), "pair_row": attn(D),
            "pair_col": attn(D), "pair_ffn": ffn(D)}


# ----------------------------------------------------------------------------
# device program
# ----------------------------------------------------------------------------

class Ctx:
    pass


def _program(nc, tc, io):
    ctx = Ctx()
    ctx.nc, ctx.tc, ctx.io = nc, tc, io

    persist = tc.alloc_tile_pool(name="persist", bufs=1)
    dram = tc.alloc_tile_pool(name="dram", bufs=1, space="DRAM")
    ps = tc.alloc_tile_pool(name="psA", bufs=3, space="PSUM")
    ps2 = tc.alloc_tile_pool(name="psB", bufs=3, space="PSUM")
    ctx.persist = persist
    ctx.ps, ctx.ps2 = ps, ps2

    pair = persist.tile([D, PT], f32, name="pair")
    msa_s = persist.tile([D, MT], f32, name="msa_s")
    nc.sync.dma_start(pair[:], io.pair_in[:])
    nc.sync.dma_start(msa_s[:], io.msa_in[:])
    ctx.msa_stream = msa_s

    W = {}
    for k, t in io.w.items():
        W[k] = persist.tile(list(t.shape), t.dtype, name="w_" + k)
        nc.sync.dma_start(W[k][:], t[:])
    ctx.W = W

    epsk = persist.tile([128, 1], f32, name="epsk")
    nc.vector.memset(epsk[:], EPS)
    ctx.eps = epsk

    cb = Ctx()
    cb.bias_send = dram.tile([NCORES, H, PT], bf16, name="bias_send")
    cb.bias_recv = dram.tile([NCORES, H, PT], bf16, name="bias_recv")
    cb.ab_send = dram.tile([NCORES, 2, CH, PR, L], bf16, name="ab_send")
    cb.ab_recv = dram.tile([NCORES, 2, CH, PR, L], bf16, name="ab_recv")
    cb.x_send = dram.tile([NCORES, CH, PR, L], bf16, name="x_send")
    cb.x_recv = dram.tile([NCORES, CH, PR, L], bf16, name="x_recv")
    cb.ab2_send = dram.tile([NCORES, 2, CH, PR, L], bf16, name="ab2_send")
    cb.ab2_recv = dram.tile([NCORES, 2, CH, PR, L], bf16, name="ab2_recv")
    cb.x2_send = dram.tile([NCORES, CH, PR, L], bf16, name="x2_send")
    cb.x2_recv = dram.tile([NCORES, CH, PR, L], bf16, name="x2_recv")
    cb.pc_send = dram.tile([NCORES, D, PR, PR], f32, name="pc_send")
    cb.pc_recv = dram.tile([NCORES, D, PR, PR], f32, name="pc_recv")
    ctx.cb = cb

    def a2a(src, dst):
        nc.gpsimd.collective_compute("AllToAll", ALU.bypass, ins=[src.opt()],
                                     outs=[dst.opt()], replica_groups=RG)

    # ---- phase B: pair bias head -> broadcast
    pbf_pool = tc.alloc_tile_pool(name="pbf", bufs=1)
    pair_bf = pbf_pool.tile([D, PT], bf16, name="pair_bf")
    nc.vector.tensor_copy(pair_bf[:], pair[:])
    sbB = tc.alloc_tile_pool(name="sbB", bufs=1)
    bias_loc = sbB.tile([H, PT], bf16, name="bias_loc")
    for g in range(PG):
        sl = slice(g * GSZ, (g + 1) * GSZ)
        pb_ps = ps2.tile([H, GSZ], f32, name="psB_t", tag="psB")
        nc.tensor.matmul(pb_ps[:], W["bias_w"][:], pair_bf[:, sl],
                         start=True, stop=True)
        nc.scalar.activation(out=bias_loc[:, sl], in_=pb_ps[:],
                             func=AF.Identity, bias=W["bias_c"][:, 0:1])
    for c in range(NCORES):
        nc.sync.dma_start(cb.bias_send[c], bias_loc[:])
    a2a(cb.bias_send, cb.bias_recv)
    sbB.release()

    go_gate = persist.tile([D, PT], bf16, name="go_gate")

    # ---- phase T1a: tri_out projections + A2A(a,b)
    sbT = tc.alloc_tile_pool(name="sbT1a", bufs=1)
    ctx.sb = sbT
    xh = _ln(ctx, None, PT, "t1", src_bf16=pair_bf)
    _gated_pair(ctx, xh, "triout", cb.ab_send, go_gate)
    a2a(cb.ab_send, cb.ab_recv)
    sbT.release()
    pbf_pool.release()

    # ---- MSA path (overlaps the A2A)
    _msa_path(ctx)

    # ---- tri_out einsum + apply
    sbE = tc.alloc_tile_pool(name="sbE1", bufs=1)
    ctx.sb = sbE
    _tri_einsum(ctx, cb.ab_recv, cb.x_send, transpose_b=False, tag="e1")
    a2a(cb.x_send, cb.x_recv)
    _tri_apply(ctx, cb.x_recv, "triout", go_gate, pair, tag="o1")
    sbE.release()

    # ---- tri_in
    sbT2 = tc.alloc_tile_pool(name="sbT2a", bufs=1)
    ctx.sb = sbT2
    xh2 = _ln(ctx, pair, PT, "t2")
    _gated_pair(ctx, xh2, "triin", cb.ab2_send, go_gate)
    a2a(cb.ab2_send, cb.ab2_recv)
    sbT2.release()
    sbE2 = tc.alloc_tile_pool(name="sbE2", bufs=1)
    ctx.sb = sbE2
    _tri_einsum(ctx, cb.ab2_recv, cb.x2_send, transpose_b=True, tag="e2")
    a2a(cb.x2_send, cb.x2_recv)
    _tri_apply(ctx, cb.x2_recv, "triin", go_gate, pair, tag="o2")
    sbE2.release()

    # ---- pair row attention
    sbP = tc.alloc_tile_pool(name="sbPr", bufs=1)
    ctx.sb = sbP
    _attention(ctx, pair, PT, "prow", n_rows=PR, strided=False, bias_tiles=None,
               tag="pr")
    sbP.release()

    # ---- reshard rows -> cols
    pview = pair[:].rearrange("d (i j) -> d i j", i=PR)
    for t in range(NCORES):
        nc.sync.dma_start(cb.pc_send[t], pview[:, :, t * PR:(t + 1) * PR])
    a2a(cb.pc_send, cb.pc_recv)
    pcol = pair  # reuse the pair tile for the column-sharded stream
    for c in range(NCORES):
        nc.sync.dma_start(pcol[:, c * PR * PR:(c + 1) * PR * PR],
                          cb.pc_recv[c].rearrange("d a b -> d (a b)"))

    # ---- column attention + FFN on pcol
    sbC = tc.alloc_tile_pool(name="sbPc", bufs=1)
    ctx.sb = sbC
    _attention(ctx, pcol, PT, "pcol", n_rows=PR, strided=True, bias_tiles=None,
               tag="pc")
    sbC.release()
    sbF = tc.alloc_tile_pool(name="sbPf", bufs=1)
    ctx.sb = sbF
    _ffn(ctx, pcol, PT, "pffn", tag="pf")
    sbF.release()

    nc.sync.dma_start(io.pair_out[:], pcol[:])

    ps2.release()
    ps.release()
    dram.release()
    persist.release()


def _ln(ctx, x_cm, n_tok, tag, src_bf16=None):
    nc, sb = ctx.nc, ctx.sb
    nt = n_tok // 128
    if src_bf16 is None:
        xb = sb.tile([D, n_tok], bf16, name=f"lncast_{tag}", tag="lncast")
        nc.vector.tensor_copy(xb[:], x_cm[:, :n_tok])
    else:
        xb = src_bf16
    xtm = sb.tile([128, nt, D], bf16, name=f"lntm_{tag}", tag="lntm")
    nc.sync.dma_start(xtm[:], xb[:, :n_tok], transpose=True)
    mv = sb.tile([128, nt, 2], f32, name=f"lnmv_{tag}", tag="lnmv")
    for t in range(nt):
        stats = sb.tile([128, 6], f32, name=f"lnst_{tag}", tag="lnst", bufs=2)
        nc.vector.bn_stats(out=stats[:], in_=xtm[:, t, :])
        nc.vector.bn_aggr(out=mv[:, t, :], in_=stats[:])
    sd = sb.tile([128, nt], f32, name=f"lnsd_{tag}", tag="lnsd")
    nc.scalar.activation(out=sd[:], in_=mv[:, :, 1], func=AF.Sqrt, bias=ctx.eps[:])
    r = sb.tile([128, nt], f32, name=f"lnr_{tag}", tag="lnr")
    nc.vector.reciprocal(out=r[:], in_=sd[:])
    for t in range(nt):
        nc.vector.tensor_scalar(out=xtm[:, t, :], in0=xtm[:, t, :],
                                scalar1=mv[:, t, 0:1], scalar2=r[:, t:t + 1],
                                op0=ALU.subtract, op1=ALU.mult)
    out = sb.tile([D, n_tok], bf16, name=f"lnout_{tag}", tag="lnout")
    nc.scalar.dma_start(out[:].rearrange("d (t f) -> d t f", f=128),
                        xtm[:].rearrange("p t f -> p (t f)"), transpose=True)
    return out


def _linear(ctx, Wt, ct, rhs, out_sl, act=None):
    nc = ctx.nc
    acc = ctx.ps.tile([128, GSZ], f32, name="psA_t", tag="psA")
    nc.tensor.matmul(acc[:Wt.shape[-1], :rhs.shape[-1]], Wt, rhs,
                     start=True, stop=True)
    if act is None:
        nc.vector.tensor_scalar_add(out_sl, acc[:Wt.shape[-1], :rhs.shape[-1]],
                                    ct)
    else:
        nc.scalar.activation(out=out_sl, in_=acc[:Wt.shape[-1], :rhs.shape[-1]],
                             func=act, bias=ct)


def _gated_pair(ctx, xh, pfx, send_buf, go_gate):
    nc, sb, W = ctx.nc, ctx.sb, ctx.W
    a_cm = sb.tile([D, PT], bf16, name=f"a_{pfx}", tag="a_cm")
    b_cm = sb.tile([D, PT], bf16, name=f"b_{pfx}", tag="b_cm")
    for g in range(PG):
        sl = slice(g * GSZ, (g + 1) * GSZ)
        for nm, gnm, dst in (("pa", "ga", a_cm), ("pb", "gb", b_cm)):
            p_t = sb.tile([D, GSZ], bf16, name=f"p_{nm}", tag="tri_p", bufs=3)
            _linear(ctx, W[f"{pfx}_{nm}_w"][:], W[f"{pfx}_{nm}_c"][:, 0:1],
                    xh[:, sl], p_t[:])
            g_t = sb.tile([D, GSZ], bf16, name=f"g_{gnm}", tag="tri_g", bufs=3)
            _linear(ctx, W[f"{pfx}_{gnm}_w"][:], W[f"{pfx}_{gnm}_c"][:, 0:1],
                    xh[:, sl], g_t[:], act=AF.Sigmoid)
            nc.vector.tensor_tensor(out=dst[:, sl], in0=p_t[:], in1=g_t[:],
                                    op=ALU.mult)
        _linear(ctx, W[f"{pfx}_go_w"][:], W[f"{pfx}_go_c"][:, 0:1],
                xh[:, sl], go_gate[:, sl], act=AF.Sigmoid)
    for c in range(NCORES):
        nc.gpsimd.dma_start(
            send_buf[c, 0].rearrange("c p l -> c (p l)"), a_cm[c * CH:(c + 1) * CH, :])
        nc.gpsimd.dma_start(
            send_buf[c, 1].rearrange("c p l -> c (p l)"), b_cm[c * CH:(c + 1) * CH, :])


def _tri_einsum(ctx, recv, x_send, transpose_b, tag):
    nc, sb = ctx.nc, ctx.sb
    aT = sb.tile([128, 2, CH, L], bf16, name=f"aT_{tag}", tag="eins_aT")
    for c in range(NCORES):
        for ch in range(CH):
            eng = nc.sync if (ch % 2 == 0) else nc.scalar
            eng.dma_start(aT[:, :, ch, c * PR:(c + 1) * PR],
                          recv[c, 0, ch], transpose=True)
    bT = sb.tile([128, 2, CH, L], bf16, name=f"bT_{tag}", tag="eins_bT")
    if transpose_b:
        for c in range(NCORES):
            for ch in range(CH):
                eng = nc.sync if (ch % 2 == 1) else nc.scalar
                eng.dma_start(bT[:, :, ch, c * PR:(c + 1) * PR],
                              recv[c, 1, ch], transpose=True)
    else:
        for c in range(NCORES):
            nc.sync.dma_start(bT[(c % 4) * PR:((c % 4) + 1) * PR, c // 4, :, :],
                              recv[c, 1].rearrange("c p l -> p c l"))
    xl = sb.tile([128, 2, CH, L], bf16, name=f"xl_{tag}", tag="eins_x")
    for ch in range(CH):
        for it in range(2):
            acc = ctx.ps2.tile([128, L], f32, name="psB_t", tag="psB")
            for kh in range(2):
                nc.tensor.matmul(acc[:], aT[:, kh, ch, it * 128:(it + 1) * 128],
                                 bT[:, kh, ch, :], start=(kh == 0), stop=(kh == 1))
            nc.vector.tensor_copy(xl[:, it, ch, :], acc[:])
    for c in range(NCORES):
        nc.gpsimd.dma_start(x_send[c].rearrange("c p l -> p c l"),
                            xl[(c % 4) * PR:((c % 4) + 1) * PR, c // 4, :, :])


def _tri_apply(ctx, x_recv, pfx, go_gate, pair, tag):
    nc, sb, W = ctx.nc, ctx.sb, ctx.W
    x_cm = sb.tile([D, PT], bf16, name=f"xcm_{tag}", tag="xcm")
    for c in range(NCORES):
        nc.gpsimd.dma_start(x_cm[c * CH:(c + 1) * CH, :],
                            x_recv[c].rearrange("c p l -> c (p l)"))
    xo = _ln(ctx, None, PT, f"lno_{tag}", src_bf16=x_cm)
    for g in range(PG):
        sl = slice(g * GSZ, (g + 1) * GSZ)
        acc = ctx.ps.tile([128, GSZ], f32, name="psA_t", tag="psA")
        nc.tensor.matmul(acc[:], W[f"{pfx}_po_w"][:], xo[:, sl],
                         start=True, stop=True)
        upd = sb.tile([D, GSZ], f32, name="po_upd", tag="po_upd", bufs=3)
        nc.vector.scalar_tensor_tensor(
            out=upd[:], in0=acc[:], scalar=W[f"{pfx}_po_c"][:, 0:1],
            op0=ALU.add, op1=ALU.mult, in1=go_gate[:, sl])
        nc.vector.tensor_tensor(out=pair[:, sl], in0=pair[:, sl], in1=upd[:],
                                op=ALU.add)


def _attention(ctx, stream, n_tok, pfx, n_rows, strided, bias_tiles, tag):
    nc, sb, W = ctx.nc, ctx.sb, ctx.W
    ng = n_tok // GSZ
    xh = _ln(ctx, stream, n_tok, f"at_{tag}")
    # heads 0-2 at bases 0/32/64 in qA; head 3 in its own base-0 tile qB
    # (PE cannot read operands based at partition 96)
    qA = sb.tile([96, n_tok], bf16, name=f"qA_{tag}", tag="qA")
    kA = sb.tile([96, n_tok], bf16, name=f"kA_{tag}", tag="kA")
    qkB = sb.tile([HD, 2, n_tok], bf16, name=f"qkB_{tag}", tag="qkB")
    qB, kB = qkB[:, 0], qkB[:, 1]
    v_cm = sb.tile([D, n_tok], bf16, name=f"v_{tag}", tag="v_cm")
    for g in range(ng):
        sl = slice(g * GSZ, (g + 1) * GSZ)
        for ci, dA, dB in ((0, qA[:], qB), (1, kA[:], kB)):
            _linear(ctx, W[f"{pfx}_qkv_w"][:, ci * D:ci * D + 96],
                    W[f"{pfx}_qkv_c"][:96, ci:ci + 1], xh[:, sl], dA[:, sl])
            _linear(ctx, W[f"{pfx}_qkv_w"][:, ci * D + 96:(ci + 1) * D],
                    W[f"{pfx}_qkv_ch"][:, ci:ci + 1], xh[:, sl], dB[:, sl])
        _linear(ctx, W[f"{pfx}_qkv_w"][:, 2 * D:3 * D],
                W[f"{pfx}_qkv_c"][:, 2:3], xh[:, sl], v_cm[:, sl])
    for i in range(n_rows):
        if not strided:
            def rview(t):
                return t[:, i * L:(i + 1) * L]
        else:
            def rview(t):
                if hasattr(t, 'tensor'):
                    return t.rearrange("d (i j) -> d j i", j=PR)[:, i, :]
                return t[:].rearrange("d (i j) -> d j i", j=PR)[:, i, :]
        if strided:
            # stage strided row into contiguous tiles (DMA needs contiguity)
            qAr = sb.tile([96, L], bf16, name="qAr", tag="qAr")
            qBr = sb.tile([HD, L], bf16, name="qBr", tag="qBr")
            kAr = sb.tile([96, L], bf16, name="kAr", tag="kAr")
            kBr = sb.tile([HD, L], bf16, name="kBr", tag="kBr")
            v_row = sb.tile([D, L], bf16, name="v_row", tag="v_row")
            nc.vector.tensor_copy(qAr[:], rview(qA))
            nc.vector.tensor_copy(qBr[:], rview(qB))
            nc.vector.tensor_copy(kAr[:], rview(kA))
            nc.vector.tensor_copy(kBr[:], rview(kB))
            nc.vector.tensor_copy(v_row[:], rview(v_cm))
            qAv, qBv, kAv, kBv, vv = qAr[:], qBr[:], kAr[:], kBr[:], v_row[:]
        else:
            qAv, qBv = rview(qA), rview(qB)
            kAv, kBv, vv = rview(kA), rview(kB), rview(v_cm)
        v_tm = sb.tile([128, 2, D], bf16, name="v_tm", tag="v_tm")
        nc.sync.dma_start(v_tm[:], vv, transpose=True)
        att4 = sb.tile([HD, H, L], bf16, name="att4", tag="att4")
        for h in range(H):
            hs = slice(h * HD, (h + 1) * HD)
            if h < 3:
                qh, kh = qAv[hs, :], kAv[hs, :]
            else:
                qh, kh = qBv, kBv
            prob2 = sb.tile([128, 2, L], bf16, name="prob2", tag="prob2", bufs=2)
            for xt in range(2):
                sc = ctx.ps.tile([128, GSZ], f32, name="psA_t", tag="psA")
                nc.tensor.matmul(sc[:, :L], qh[:, xt * 128:(xt + 1) * 128],
                                 kh, start=True, stop=True)
                if bias_tiles is not None:
                    nc.vector.tensor_tensor(out=sc[:, :L], in0=sc[:, :L],
                                            in1=bias_tiles[h][xt][:], op=ALU.add)
                rs = sb.tile([128, 1], f32, name="rs", tag="rs", bufs=2)
                nc.scalar.activation(out=prob2[:, xt, :], in_=sc[:, :L],
                                     func=AF.Exp, accum_out=rs[:])
                nc.vector.reciprocal(out=rs[:], in_=rs[:])
                nc.vector.tensor_scalar_mul(prob2[:, xt, :], prob2[:, xt, :],
                                            rs[:])
            # pT layout [yk 128, xt 2, kt 2, xq 128]: one batched transpose
            pT = sb.tile([128, 2, 2, 128], bf16, name="pT", tag="pT", bufs=2)
            nc.sync.dma_start(pT[:], prob2[:].rearrange("p x l -> p (x l)"),
                               transpose=True)
            av_ps = ctx.ps2.tile([HD, L], f32, name="psB_t", tag="psB")
            for kt in range(2):
                nc.tensor.matmul(av_ps[:], v_tm[:, kt, hs],
                                 pT[:, :, kt, :], start=(kt == 0),
                                 stop=(kt == 1))
            nc.scalar.copy(out=att4[:, h, :], in_=av_ps[:])
        acc = ctx.ps.tile([128, GSZ], f32, name="psA_t", tag="psA")
        for h in range(H):
            nc.tensor.matmul(acc[:, :L], W[f"{pfx}_proj_w"][:, h, :],
                             att4[:, h, :], start=(h == 0), stop=(h == H - 1))
        nc.vector.scalar_tensor_tensor(
            out=rview(stream), in0=acc[:, :L],
            scalar=W[f"{pfx}_proj_c"][:, 0:1],
            op0=ALU.add, op1=ALU.add, in1=rview(stream))


def _ffn(ctx, stream, n_tok, pfx, tag):
    nc, sb, W = ctx.nc, ctx.sb, ctx.W
    ng = n_tok // GSZ
    xh = _ln(ctx, stream, n_tok, f"ffn_{tag}")
    hid = [sb.tile([D, n_tok], bf16, name=f"hid{kc}_{tag}", tag=f"hid{kc}")
           for kc in range(4)]
    for g in range(ng):
        sl = slice(g * GSZ, (g + 1) * GSZ)
        for kc in range(4):
            _linear(ctx, W[f"{pfx}_w1_w"][:, kc * D:(kc + 1) * D],
                    W[f"{pfx}_w1_c"][:, kc:kc + 1], xh[:, sl], hid[kc][:, sl],
                    act=(AF.Tanh if GELU_SUB[0] else AF.Gelu))
        acc = ctx.ps.tile([128, GSZ], f32, name="psA_t", tag="psA")
        for kc in range(4):
            nc.tensor.matmul(acc[:], W[f"{pfx}_w2_w"][:, kc, :], hid[kc][:, sl],
                             start=(kc == 0), stop=(kc == 3))
        nc.vector.scalar_tensor_tensor(
            out=stream[:, sl], in0=acc[:], scalar=W[f"{pfx}_w2_c"][:, 0:1],
            op0=ALU.add, op1=ALU.add, in1=stream[:, sl])


def _msa_path(ctx):
    nc, cb = ctx.nc, ctx.cb
    sb = ctx.tc.alloc_tile_pool(name="sbM", bufs=1)
    ctx.sb = sb
    bt_f32 = []
    for h in range(H):
        row = []
        for xt in range(2):
            bb = sb.tile([128, L], bf16, name=f"bb_{h}_{xt}", tag="biasb")
            for cc in range(4):
                c = 4 * xt + cc
                nc.sync.dma_start(
                    bb[cc * PR:(cc + 1) * PR, :],
                    cb.bias_recv[c, h].rearrange("(x y) -> x y", y=L))
            bf = sb.tile([128, L], f32, name=f"biasf_{h}_{xt}", tag=f"biasf_{h}_{xt}")
            nc.vector.tensor_copy(bf[:], bb[:])
            row.append(bf)
        bt_f32.append(row)
    _attention(ctx, ctx.msa_stream, MT, "msa", n_rows=MN, strided=False,
               bias_tiles=bt_f32, tag="m1")
    _ffn(ctx, ctx.msa_stream, MT, "msaffn", tag="m2")
    nc.sync.dma_start(ctx.io.msa_out[:], ctx.msa_stream[:])
    sb.release()


GELU_SUB = [False]  # sim-only: replace Gelu (unimplemented in CoreSim) with Tanh


def build():
    nc = bacc.Bacc()
    P = _prep_params(_dummy_params())
    io = Ctx()
    io.pair_in = nc.dram_tensor("pair_in", [D, PT], f32, kind="ExternalInput")
    io.msa_in = nc.dram_tensor("msa_in", [D, MT], f32, kind="ExternalInput")
    io.w = {}
    for k, v in P.items():
        dt = bf16 if v.dtype == _BF else f32
        io.w[k] = nc.dram_tensor(k, list(v.shape), dt, kind="ExternalInput")
    io.pair_out = nc.dram_tensor("pair_out", [D, PT], f32, kind="ExternalOutput")
    io.msa_out = nc.dram_tensor("msa_out", [D, MT], f32, kind="ExternalOutput")

    with tile.TileContext(nc) as tc:
        _program(nc, tc, io)
    nc.finalize()
    return nc


_CACHED = {}


def kernel(msa, pair, params):
    msa = np.asarray(msa)
    pair = np.asarray(pair)
    P = _prep_params(params)

    if "nc" not in _CACHED:
        _CACHED["nc"] = build()
    nc = _CACHED["nc"]

    in_maps = []
    for c in range(NCORES):
        m = {}
        pr = pair[0, c * PR:(c + 1) * PR, :, :]
        m["pair_in"] = np.ascontiguousarray(
            pr.transpose(2, 0, 1).reshape(D, PT)).astype(np.float32)
        ms = msa[0, c * MN:(c + 1) * MN, :, :]
        m["msa_in"] = np.ascontiguousarray(
            ms.transpose(2, 0, 1).reshape(D, MT)).astype(np.float32)
        m.update(P)
        in_maps.append(m)

    res = run_bass_kernel_spmd(nc, in_maps, core_ids=list(range(NCORES)))

    msa_out = np.zeros((B, N, L, D), np.float32)
    pair_out = np.zeros((B, L, L, D), np.float32)
    for c in range(NCORES):
        mo = res.results[c]["msa_out"].reshape(D, MN, L).transpose(1, 2, 0)
        msa_out[0, c * MN:(c + 1) * MN] = mo
        po = res.results[c]["pair_out"].reshape(D, L, PR).transpose(1, 2, 0)
        pair_out[0, :, c * PR:(c + 1) * PR, :] = po
    return msa_out, pair_out
